# revision 18
# baseline (speedup 1.0000x reference)
"""Trainium2 Bass kernel for nn_CNN_MoE_v1 (moe_routing).

Strategy: data-parallel over batch across 8 NeuronCores (8 samples/core).
Per core, on device:
  - gating (fp32): channel-mean -> normalized prototype distances ->
    softmax -> top-2 (value + index, lax.top_k tie semantics)
  - expert weights fetched per sample-slot with dma_gather (indices
    computed on device), weights pre-laid-out on host as one table
    row-per-(expert, contract-partition)
  - 3x3 conv as 18 accumulated fp32r matmuls (9 shifts x 2 c-blocks) per
    output half, 1x1 conv as 3x2 fp32r matmuls reading x + new features
  - experts_out written per (slot, o-block, half); out = sum of
    score-weighted expert outputs (DVE)
All five reference outputs are produced: (out, scores, idxs, query_fm,
experts_out).
"""

import sys

for _p in ('/opt/trn_rl_repo', '/root/.axon_site/_ro/trn_rl_repo'):
    if _p not in sys.path:
        try:
            import concourse  # noqa: F401
            break
        except Exception:
            sys.path.insert(0, _p)

import numpy as np

import concourse.bass as bass
import concourse.mybir as mybir
import concourse.tile as tile
from concourse import bacc
from concourse.bass_utils import run_bass_kernel_spmd

F32 = mybir.dt.float32
F32R = mybir.dt.float32r
BF16 = mybir.dt.bfloat16
FP16 = mybir.dt.float16
I32 = mybir.dt.int32
I16 = mybir.dt.int16
AX = mybir.AxisListType
OP = mybir.AluOpType
ACT = mybir.ActivationFunctionType

B, C, S, K, E = 64, 256, 28, 128, 16
NCORES = 8
BS = B // NCORES          # samples per core
HW = S * S                # 784
SP = S + 2                # 30 (padded)
HALF = HW // 2            # 392
W3_COLS = 9 * 2 * 128     # 2304
W1_COLS = 3 * 2 * 128     # 768
WROW = W3_COLS + W1_COLS  # 3072

_cache = {}


def _build_program():
    nc = bacc.Bacc()

    x_in = nc.declare_dram_parameter("x_in", [BS, C, SP * SP], F32R, isOutput=False)
    wtab = nc.declare_dram_parameter("wtab", [E * 128, WROW], BF16, isOutput=False)
    protoT = nc.declare_dram_parameter("protoT", [128, 7 * E], F32, isOutput=False)
    pn2 = nc.declare_dram_parameter("pn2", [BS, E], F32, isOutput=False)
    base16 = nc.declare_dram_parameter("base16", [128, 8], F32, isOutput=False)
    iota16 = nc.declare_dram_parameter("iota16", [BS, E], F32, isOutput=False)
    iota1e9 = nc.declare_dram_parameter("iota1e9", [BS, E], F32, isOutput=False)
    ident = nc.declare_dram_parameter("ident", [128, 128], F32, isOutput=False)
    ones1 = nc.declare_dram_parameter("ones1", [1, 128], F32, isOutput=False)
    cmean = nc.declare_dram_parameter("cmean", [128, 1], F32R, isOutput=False)

    out_d = nc.declare_dram_parameter("out_d", [BS, C, HW], F32, isOutput=True)
    scores_d = nc.declare_dram_parameter("scores_d", [BS, E], F32, isOutput=True)
    idxs_d = nc.declare_dram_parameter("idxs_d", [BS, 2], I32, isOutput=True)
    qf_d = nc.declare_dram_parameter("qf_d", [BS, HW], F32, isOutput=True)
    eo_d = nc.declare_dram_parameter("eo_d", [BS, 2, C, HW], F32, isOutput=True)

    with tile.TileContext(nc) as tc:
        with tc.tile_pool(name="const", bufs=1) as cp, \
             tc.tile_pool(name="xp", bufs=1) as xp, \
             tc.tile_pool(name="xgp", bufs=4) as xgp, \
             tc.tile_pool(name="gate", bufs=1) as gp:

            # ---- prefetch sample 0 gating-x before anything else ----
            xg0 = xgp.tile([128, 2, SP * SP], F32R, tag="xg")
            for cb in range(2):
                for hh in range(2):
                    nc.sync.dma_start(
                        xg0[:, cb, hh * 450:(hh + 1) * 450],
                        x_in[0, cb * 128:(cb + 1) * 128,
                             hh * 450:(hh + 1) * 450])
            protoT_t = cp.tile([128, 7, E], F32)
            nc.sync.dma_start(protoT_t[:], protoT[:].rearrange("p (j e) -> p j e", j=7))
            pn2_t = cp.tile([BS, E], F32)
            nc.sync.dma_start(pn2_t[:], pn2[:])
            base16_t = cp.tile([128, 8], F32)
            nc.sync.dma_start(base16_t[:], base16[:])
            iota16_t = cp.tile([BS, E], F32)
            nc.sync.dma_start(iota16_t[:], iota16[:])
            iota1e9_t = cp.tile([BS, E], F32)
            nc.sync.dma_start(iota1e9_t[:], iota1e9[:])
            ident_t = cp.tile([128, 128], F32)
            nc.sync.dma_start(ident_t[:], ident[:])
            ones1_t = cp.tile([1, 128], F32)
            nc.sync.dma_start(ones1_t[:], ones1[:])
            cmean_t = cp.tile([128, 1], F32R)
            nc.sync.dma_start(cmean_t[:], cmean[:])

            # ---- x load into zero-padded [128, b, cb, 30*30] ----
            xbf = xp.tile([128, BS, 2, SP * SP], BF16)

            def xviewb(b, cb, row0, col0, nrows):
                """[128, nrows, 28] strided bf16 view (convs)."""
                return xbf[:, b, cb, :].rearrange("p (r c) -> p r c", r=SP)[
                    :, row0:row0 + nrows, col0:col0 + S]

            # ---- gating ----
            qf8 = gp.tile([BS, 896], F32)
            _ = None
            nc.vector.memset(qf8[:, 784:], 0.0)

            with tc.tile_pool(name="gps", bufs=2, space="PSUM") as gps, \
                 tc.tile_pool(name="gps2", bufs=1, space="PSUM") as gps2:
                for b in range(BS):
                    if b == 0:
                        xg = xg0
                    else:
                        xg = xgp.tile([128, 2, SP * SP], F32R, tag="xg")
                        for cb in range(2):
                            for hh in range(2):
                                nc.sync.dma_start(
                                    xg[:, cb, hh * 450:(hh + 1) * 450],
                                    x_in[b, cb * 128:(cb + 1) * 128,
                                         hh * 450:(hh + 1) * 450])
                    qf_ps = gps.tile([1, 1024], F32, tag="qf")
                    for half in range(2):
                        o0 = 0 if half == 0 else 512
                        out_ap = qf_ps[0:1, o0:o0 + HALF]
                        for cb in range(2):
                            rhs = xg[:, cb, :].rearrange(
                                "p (r c) -> p r c", r=SP)[
                                :, 1 + half * 14:15 + half * 14, 1:1 + S]
                            nc.tensor.matmul(out_ap, cmean_t[:, 0:1], rhs,
                                             start=(cb == 0), stop=(cb == 1))
                    nc.scalar.copy(xbf[:, b, :, :], xg[:])
                    qftmp = gp.tile([1, HW], F32, tag="qftmp")
                    nc.vector.tensor_copy(
                        qftmp[0:1, :].rearrange("p (c q) -> p c q", c=2),
                        qf_ps[0:1, :].rearrange("p (c q) -> p c q", c=2)[:, :, 0:HALF])
                    nc.sync.dma_start(qf8[b:b + 1, 0:HW], qftmp[0:1, :])

                nc.sync.dma_start(qf_d[:], qf8[0:BS, 0:HW])

                # qn2 = sum(qf^2); inv = 1/max(sqrt(qn2),1e-12)
                sq = gp.tile([BS, HW], F32)
                qn2 = gp.tile([BS, 1], F32)
                nc.scalar.activation(sq[:], qf8[0:BS, 0:HW], ACT.Square,
                                     accum_out=qn2[:])
                nrm = gp.tile([BS, 1], F32)
                nc.scalar.sqrt(nrm[:], qn2[:])
                nrmc = gp.tile([BS, 1], F32)
                nc.vector.tensor_scalar_max(nrmc[:], nrm[:], 1e-12)
                inv = gp.tile([BS, 1], F32)
                nc.vector.reciprocal(inv[:], nrmc[:])
                qn2i = gp.tile([BS, 1], F32)
                nc.vector.tensor_scalar(qn2i[:], qn2[:], inv[:, 0:1], inv[:, 0:1],
                                        OP.mult, OP.mult)

                # qfT via PE transpose: [8, 128] -> [128, 8] per 128-chunk
                qfT = gp.tile([128, 7, BS], F32)
                for j in range(7):
                    tp = gps.tile([128, BS], F32, tag="tp")
                    nc.tensor.transpose(tp[:], qf8[0:BS, j * 128:(j + 1) * 128],
                                        ident_t[0:BS, 0:BS])
                    nc.vector.tensor_copy(qfT[:, j, :], tp[:])

                # SS[b,e] = qf . proto_e
                ss_ps = gps2.tile([BS, E], F32, tag="ss")
                for j in range(7):
                    nc.tensor.matmul(ss_ps[:], qfT[:, j, :], protoT_t[:, j, :],
                                     start=(j == 0), stop=(j == 6))

                # d2 = qn2*inv^2 + pn2 - 2*SS*inv ; d = sqrt(max(d2,0))
                t1 = gp.tile([BS, E], F32)
                nc.vector.tensor_scalar(t1[:], ss_ps[:], inv[:, 0:1], -2.0,
                                        OP.mult, OP.mult)
                t2 = gp.tile([BS, E], F32)
                nc.vector.tensor_scalar(t2[:], t1[:], qn2i[:, 0:1], None, OP.add)
                d2 = gp.tile([BS, E], F32)
                nc.vector.tensor_tensor(d2[:], t2[:], pn2_t[:], OP.add)
                d2c = gp.tile([BS, E], F32)
                nc.vector.tensor_scalar_max(d2c[:], d2[:], 0.0)
                dd = gp.tile([BS, E], F32)
                nc.scalar.sqrt(dd[:], d2c[:])

                # softmax over 16
                mx = gp.tile([BS, 1], F32)
                nc.vector.tensor_reduce(mx[:], dd[:], axis=AX.X, op=OP.max)
                negm = gp.tile([BS, 1], F32)
                nc.vector.tensor_scalar_mul(negm[:], mx[:], -1.0)
                exps = gp.tile([BS, E], F32)
                sumexp = gp.tile([BS, 1], F32)
                nc.scalar.activation(exps[:], dd[:], ACT.Exp, bias=negm[:, 0:1],
                                     accum_out=sumexp[:])
                rsum = gp.tile([BS, 1], F32)
                nc.vector.reciprocal(rsum[:], sumexp[:])
                sc = gp.tile([BS, E], F32)
                nc.vector.tensor_scalar(sc[:], exps[:], rsum[:, 0:1], None, OP.mult)
                nc.sync.dma_start(scores_d[:], sc[:])

                # top-2 via DVE top-8 sort
                mx8 = gp.tile([BS, 8], F32)
                nc.vector.max(mx8[:], sc[:])
                ix8 = gp.tile([BS, 8], mybir.dt.uint32)
                nc.vector.max_index(ix8[:], mx8[:], sc[:])
                idxf = gp.tile([BS, 2], F32)
                nc.vector.tensor_copy(idxf[:], ix8[:, 0:2])
                idxi = gp.tile([BS, 2], I32)
                nc.vector.tensor_copy(idxi[:], ix8[:, 0:2])
                nc.sync.dma_start(idxs_d[:], idxi[:])
                msel = gp.tile([BS, 2], F32)
                nc.vector.tensor_copy(msel[:], mx8[:, 0:2])
                e128 = gp.tile([BS, 2], F32)
                nc.vector.tensor_scalar(e128[:], idxf[:], 128.0, None, OP.mult)

                # pack [8,2]+[8,2] to one partition, broadcast to 128 partitions
                bsrc = gp.tile([1, 32], F32)
                nc.sync.dma_start(
                    bsrc[0:1, 0:16].rearrange("p (b i) -> p b i", b=BS), e128[:])
                nc.sync.dma_start(
                    bsrc[0:1, 16:32].rearrange("p (b i) -> p b i", b=BS), msel[:])
                bc_ps = gps2.tile([128, 32], F32, tag="bc")
                nc.tensor.matmul(bc_ps[:], ones1_t[0:1, :], bsrc[0:1, :],
                                 start=True, stop=True)
                msb = cp.tile([128, 16], F32)
                nc.vector.tensor_copy(msb[:], bc_ps[:, 16:32])
                e128b = gp.tile([128, 16], F32)
                nc.vector.tensor_copy(e128b[:], bc_ps[:, 0:16])

                gidxf = gp.tile([128, 16, 8], F32)
                for si in range(16):
                    nc.vector.tensor_scalar(gidxf[:, si, :], base16_t[:],
                                            e128b[:, si:si + 1], None, OP.add)
                gidx = cp.tile([128, 16 * 8], I16)
                nc.vector.tensor_copy(gidx[:], gidxf[:].rearrange("p a b -> p (a b)"))

            # ---- main expert-conv loop ----
            with tc.tile_pool(name="wp", bufs=3) as wp, \
                 tc.tile_pool(name="nfp", bufs=3) as nfp, \
                 tc.tile_pool(name="eop", bufs=6) as eop, \
                 tc.tile_pool(name="accp", bufs=2) as accp, \
                 tc.tile_pool(name="cps", bufs=2, space="PSUM") as cps, \
                 tc.tile_pool(name="cps1", bufs=2, space="PSUM") as cps1:
                GROUPS = [(0, 1), (1, 1), (2, 2), (4, 4), (8, 4), (12, 4)]
                slot_map = {}
                wgroups = []
                for g, (st, sz) in enumerate(GROUPS):
                    wg = wp.tile([128, 4, WROW], BF16, tag="w")
                    nc.gpsimd.dma_gather(wg[:, 0:sz, :], wtab[:],
                                         gidx[:, st * 8:(st + sz) * 8],
                                         sz * 128, sz * 128, WROW)
                    wgroups.append(wg)
                    for k in range(sz):
                        slot_map[st + k] = (g, k)
                for b in range(BS):
                    acc = accp.tile([128, 2, 2, HALF], F32, tag="acc")
                    for i in range(2):
                        si = b * 2 + i
                        g, k = slot_map[si]
                        wsb = wgroups[g][:, k:k + 1, :]
                        nf = nfp.tile([128, 2, HALF], BF16, tag="nf")
                        ps3 = cps.tile([128, 2, 512], F32, tag="ps3")
                        j = 0
                        for dy in range(3):
                            for dx in range(3):
                                for cb in range(2):
                                    w_ap = wsb[:, 0, j * 128:(j + 1) * 128]
                                    for half in range(2):
                                        rhs = xviewb(b, cb, half * 14 + dy, dx, 14)
                                        nc.tensor.matmul(
                                            ps3[:, half, 0:HALF], w_ap,
                                            rhs, start=(j == 0), stop=(j == 17))
                                    j += 1
                        for half in range(2):
                            nc.vector.tensor_copy(nf[:, half, :],
                                                  ps3[:, half, 0:HALF])

                        for ob in range(2):
                            ps1 = cps1.tile([128, 2, 512], F32, tag="ps1")
                            for ib in range(3):
                                w0 = W3_COLS + (ib * 2 + ob) * 128
                                w_ap = wsb[:, 0, w0:w0 + 128]
                                for half in range(2):
                                    if ib < 2:
                                        rhs = xviewb(b, ib, 1 + half * 14, 1, 14)
                                    else:
                                        rhs = nf[:, half, :]
                                    nc.tensor.matmul(
                                        ps1[:, half, 0:HALF], w_ap, rhs,
                                        start=(ib == 0), stop=(ib == 2))
                            for half in range(2):
                                eo = eop.tile([128, HALF], F32, tag="eo")
                                nc.vector.tensor_copy(eo[:], ps1[:, half, 0:HALF])
                                nc.sync.dma_start(
                                    eo_d[b, i, ob * 128:(ob + 1) * 128,
                                         half * HALF:(half + 1) * HALF], eo[:])
                                if i == 0:
                                    nc.vector.tensor_scalar(
                                        acc[:, ob, half, :], eo[:],
                                        msb[:, si:si + 1], None, OP.mult)
                                else:
                                    tmp = eop.tile([128, HALF], F32, tag="tmp")
                                    nc.vector.tensor_scalar(
                                        tmp[:], eo[:], msb[:, si:si + 1], None,
                                        OP.mult)
                                    nc.vector.tensor_tensor(
                                        acc[:, ob, half, :], acc[:, ob, half, :],
                                        tmp[:], OP.add)
                    for ob in range(2):
                        nc.sync.dma_start(
                            out_d[b, ob * 128:(ob + 1) * 128, :]
                            .rearrange("p (h q) -> p h q", h=2),
                            acc[:, ob, :, :])

    nc.finalize()
    return nc


def _host_prep(csp, ccp, proto):
    w3 = csp.transpose(0, 3, 4, 2, 1).reshape(E, 3, 3, 2, 128, K)
    w3 = w3.transpose(0, 4, 1, 2, 3, 5).reshape(E, 128, W3_COLS)
    w1 = ccp[:, :, :, 0, 0].reshape(E, 2, 128, 3, 128)
    w1 = w1.transpose(0, 4, 3, 1, 2).reshape(E, 128, W1_COLS)
    import ml_dtypes
    wtab = np.concatenate([w3, w1], axis=2).reshape(E * 128, WROW)
    wtab = np.ascontiguousarray(wtab.astype(ml_dtypes.bfloat16))

    protoT = np.zeros((128, 7, E), np.float32)
    for j in range(7):
        seg = proto[:, j * 128:min((j + 1) * 128, HW)]
        protoT[:seg.shape[1], j, :] = seg.T
    protoT = protoT.reshape(128, 7 * E)

    pn2 = np.broadcast_to((proto.astype(np.float32) ** 2).sum(axis=1)[None, :],
                          (BS, E)).copy()
    base16 = (np.arange(8)[None, :] * 16 + (np.arange(128) % 16)[:, None]).astype(
        np.float32)
    iota16 = np.broadcast_to(np.arange(E, dtype=np.float32)[None, :], (BS, E)).copy()
    iota1e9 = iota16 + 1e4
    ident = np.eye(128, dtype=np.float32)
    ones1 = np.ones((1, 128), np.float32)
    cmean = np.full((128, 1), 1.0 / C, np.float32)
    return dict(wtab=wtab, protoT=protoT, pn2=pn2, base16=base16, iota16=iota16,
                iota1e9=iota1e9, ident=ident, ones1=ones1, cmean=cmean)


def kernel(pretrained_x, x, conv_special_param, conv_channel_param, prototype,
           topk):
    assert int(topk) == 2
    x = np.asarray(x, np.float32).reshape(B, C, S, S)
    xpad = np.zeros((B, C, SP, SP), np.float32)
    xpad[:, :, 1:1 + S, 1:1 + S] = x
    xpad = xpad.reshape(B, C, SP * SP)

    consts = _host_prep(np.asarray(conv_special_param, np.float32),
                        np.asarray(conv_channel_param, np.float32),
                        np.asarray(prototype, np.float32))

    if "nc" not in _cache:
        _cache["nc"] = _build_program()
    nc = _cache["nc"]

    in_maps = []
    for c in range(NCORES):
        m = dict(consts)
        m["x_in"] = xpad[c * BS:(c + 1) * BS]
        in_maps.append(m)

    res = run_bass_kernel_spmd(nc, in_maps, core_ids=list(range(NCORES)),
                               **_cache.get("run_kwargs", {}))
    kernel.last_results = res

    outs = res.results
    out = np.concatenate([r["out_d"] for r in outs]).reshape(B, C, S, S)
    scores = np.concatenate([r["scores_d"] for r in outs])
    idxs = np.concatenate([r["idxs_d"] for r in outs]).astype(np.int32)
    query_fm = np.concatenate([r["qf_d"] for r in outs])
    experts_out = np.concatenate([r["eo_d"] for r in outs]).reshape(B, 2, C, S, S)
    return out, scores, idxs, query_fm, experts_out


# revision 19
# speedup vs baseline: 1.0331x; 1.0331x over previous
"""Trainium2 Bass kernel for nn_CNN_MoE_v1 (moe_routing).

Strategy: data-parallel over batch across 8 NeuronCores (8 samples/core).
Per core, on device:
  - gating (fp32): channel-mean -> normalized prototype distances ->
    softmax -> top-2 (value + index, lax.top_k tie semantics)
  - expert weights fetched per sample-slot with dma_gather (indices
    computed on device), weights pre-laid-out on host as one table
    row-per-(expert, contract-partition)
  - 3x3 conv as 18 accumulated fp32r matmuls (9 shifts x 2 c-blocks) per
    output half, 1x1 conv as 3x2 fp32r matmuls reading x + new features
  - experts_out written per (slot, o-block, half); out = sum of
    score-weighted expert outputs (DVE)
All five reference outputs are produced: (out, scores, idxs, query_fm,
experts_out).
"""

import sys

for _p in ('/opt/trn_rl_repo', '/root/.axon_site/_ro/trn_rl_repo'):
    if _p not in sys.path:
        try:
            import concourse  # noqa: F401
            break
        except Exception:
            sys.path.insert(0, _p)

import numpy as np

import concourse.bass as bass
import concourse.mybir as mybir
import concourse.tile as tile
from concourse import bacc
from concourse.bass_utils import run_bass_kernel_spmd

F32 = mybir.dt.float32
F32R = mybir.dt.float32r
BF16 = mybir.dt.bfloat16
FP16 = mybir.dt.float16
I32 = mybir.dt.int32
I16 = mybir.dt.int16
AX = mybir.AxisListType
OP = mybir.AluOpType
ACT = mybir.ActivationFunctionType

B, C, S, K, E = 64, 256, 28, 128, 16
NCORES = 8
BS = B // NCORES          # samples per core
HW = S * S                # 784
SP = S + 2                # 30 (padded)
HALF = HW // 2            # 392
W3_COLS = 9 * 2 * 128     # 2304
W1_COLS = 3 * 2 * 128     # 768
WROW = W3_COLS + W1_COLS  # 3072

_cache = {}


def _build_program():
    nc = bacc.Bacc()

    x_in = nc.declare_dram_parameter("x_in", [BS, C, SP * SP], F32R, isOutput=False)
    wtab = nc.declare_dram_parameter("wtab", [E * 128, WROW], BF16, isOutput=False)
    protoT = nc.declare_dram_parameter("protoT", [128, 7 * E], F32, isOutput=False)
    pn2 = nc.declare_dram_parameter("pn2", [BS, E], F32, isOutput=False)
    base16 = nc.declare_dram_parameter("base16", [128, 8], F32, isOutput=False)
    iota16 = nc.declare_dram_parameter("iota16", [BS, E], F32, isOutput=False)
    iota1e9 = nc.declare_dram_parameter("iota1e9", [BS, E], F32, isOutput=False)
    ident = nc.declare_dram_parameter("ident", [128, 128], F32, isOutput=False)
    ones1 = nc.declare_dram_parameter("ones1", [1, 128], F32, isOutput=False)
    cmean = nc.declare_dram_parameter("cmean", [128, 1], F32R, isOutput=False)

    out_d = nc.declare_dram_parameter("out_d", [BS, C, HW], F32, isOutput=True)
    scores_d = nc.declare_dram_parameter("scores_d", [BS, E], F32, isOutput=True)
    idxs_d = nc.declare_dram_parameter("idxs_d", [BS, 2], I32, isOutput=True)
    qf_d = nc.declare_dram_parameter("qf_d", [BS, HW], F32, isOutput=True)
    eo_d = nc.declare_dram_parameter("eo_d", [BS, 2, C, HW], F32, isOutput=True)

    with tile.TileContext(nc) as tc:
        with tc.tile_pool(name="const", bufs=1) as cp, \
             tc.tile_pool(name="xp", bufs=1) as xp, \
             tc.tile_pool(name="xgp", bufs=8) as xgp, \
             tc.tile_pool(name="gate", bufs=1) as gp, \
             tc.tile_pool(name="qtp", bufs=3) as qtp:

            # ---- prefetch sample 0 gating-x before anything else ----
            xg0 = xgp.tile([128, 2, SP * SP], F32R, tag="xg")
            for cb in range(2):
                nc.sync.dma_start(xg0[:, cb, :],
                                  x_in[0, cb * 128:(cb + 1) * 128, :])
            protoT_t = cp.tile([128, 7, E], F32)
            nc.sync.dma_start(protoT_t[:], protoT[:].rearrange("p (j e) -> p j e", j=7))
            pn2_t = cp.tile([BS, E], F32)
            nc.sync.dma_start(pn2_t[:], pn2[:])
            base16_t = cp.tile([128, 8], F32)
            nc.sync.dma_start(base16_t[:], base16[:])
            iota16_t = cp.tile([BS, E], F32)
            nc.sync.dma_start(iota16_t[:], iota16[:])
            iota1e9_t = cp.tile([BS, E], F32)
            nc.sync.dma_start(iota1e9_t[:], iota1e9[:])
            ident_t = cp.tile([128, 128], F32)
            nc.sync.dma_start(ident_t[:], ident[:])
            ones1_t = cp.tile([1, 128], F32)
            nc.sync.dma_start(ones1_t[:], ones1[:])
            cmean_t = cp.tile([128, 1], F32R)
            nc.sync.dma_start(cmean_t[:], cmean[:])

            # ---- x load into zero-padded [128, b, cb, 30*30] ----
            xbf = xp.tile([128, BS, 2, SP * SP], BF16)

            def xviewb(b, cb, row0, col0, nrows):
                """[128, nrows, 28] strided bf16 view (convs)."""
                return xbf[:, b, cb, :].rearrange("p (r c) -> p r c", r=SP)[
                    :, row0:row0 + nrows, col0:col0 + S]

            # ---- gating ----
            qf8 = gp.tile([BS, 896], F32)
            _ = None
            nc.vector.memset(qf8[:, 784:], 0.0)

            with tc.tile_pool(name="gps", bufs=2, space="PSUM") as gps, \
                 tc.tile_pool(name="gps2", bufs=1, space="PSUM") as gps2:
                for b in range(BS):
                    if b == 0:
                        xg = xg0
                    else:
                        xg = xgp.tile([128, 2, SP * SP], F32R, tag="xg")
                        for cb in range(2):
                            nc.sync.dma_start(
                                xg[:, cb, :],
                                x_in[b, cb * 128:(cb + 1) * 128, :])
                    qf_ps = gps.tile([1, 1024], F32, tag="qf")
                    for half in range(2):
                        o0 = 0 if half == 0 else 512
                        out_ap = qf_ps[0:1, o0:o0 + HALF]
                        for cb in range(2):
                            rhs = xg[:, cb, :].rearrange(
                                "p (r c) -> p r c", r=SP)[
                                :, 1 + half * 14:15 + half * 14, 1:1 + S]
                            nc.tensor.matmul(out_ap, cmean_t[:, 0:1], rhs,
                                             start=(cb == 0), stop=(cb == 1))
                    nc.scalar.copy(xbf[:, b, :, :], xg[:])
                    qftmp = qtp.tile([1, HW], F32, tag="qftmp")
                    nc.vector.tensor_copy(
                        qftmp[0:1, :].rearrange("p (c q) -> p c q", c=2),
                        qf_ps[0:1, :].rearrange("p (c q) -> p c q", c=2)[:, :, 0:HALF])
                    nc.sync.dma_start(qf8[b:b + 1, 0:HW], qftmp[0:1, :])

                nc.sync.dma_start(qf_d[:], qf8[0:BS, 0:HW])

                # qn2 = sum(qf^2); inv = 1/max(sqrt(qn2),1e-12)
                sq = gp.tile([BS, HW], F32)
                qn2 = gp.tile([BS, 1], F32)
                nc.scalar.activation(sq[:], qf8[0:BS, 0:HW], ACT.Square,
                                     accum_out=qn2[:])
                nrm = gp.tile([BS, 1], F32)
                nc.scalar.sqrt(nrm[:], qn2[:])
                nrmc = gp.tile([BS, 1], F32)
                nc.vector.tensor_scalar_max(nrmc[:], nrm[:], 1e-12)
                inv = gp.tile([BS, 1], F32)
                nc.vector.reciprocal(inv[:], nrmc[:])
                qn2i = gp.tile([BS, 1], F32)
                nc.vector.tensor_scalar(qn2i[:], qn2[:], inv[:, 0:1], inv[:, 0:1],
                                        OP.mult, OP.mult)

                # qfT via PE transpose: [8, 128] -> [128, 8] per 128-chunk
                qfT = gp.tile([128, 7, BS], F32)
                for j in range(7):
                    tp = gps.tile([128, BS], F32, tag="tp")
                    nc.tensor.transpose(tp[:], qf8[0:BS, j * 128:(j + 1) * 128],
                                        ident_t[0:BS, 0:BS])
                    nc.vector.tensor_copy(qfT[:, j, :], tp[:])

                # SS[b,e] = qf . proto_e
                ss_ps = gps2.tile([BS, E], F32, tag="ss")
                for j in range(7):
                    nc.tensor.matmul(ss_ps[:], qfT[:, j, :], protoT_t[:, j, :],
                                     start=(j == 0), stop=(j == 6))

                # d2 = qn2*inv^2 + pn2 - 2*SS*inv ; d = sqrt(max(d2,0))
                t1 = gp.tile([BS, E], F32)
                nc.vector.tensor_scalar(t1[:], ss_ps[:], inv[:, 0:1], -2.0,
                                        OP.mult, OP.mult)
                t2 = gp.tile([BS, E], F32)
                nc.vector.tensor_scalar(t2[:], t1[:], qn2i[:, 0:1], None, OP.add)
                d2 = gp.tile([BS, E], F32)
                nc.vector.tensor_tensor(d2[:], t2[:], pn2_t[:], OP.add)
                d2c = gp.tile([BS, E], F32)
                nc.vector.tensor_scalar_max(d2c[:], d2[:], 0.0)
                dd = gp.tile([BS, E], F32)
                nc.scalar.sqrt(dd[:], d2c[:])

                # softmax over 16
                mx = gp.tile([BS, 1], F32)
                nc.vector.tensor_reduce(mx[:], dd[:], axis=AX.X, op=OP.max)
                negm = gp.tile([BS, 1], F32)
                nc.vector.tensor_scalar_mul(negm[:], mx[:], -1.0)
                exps = gp.tile([BS, E], F32)
                sumexp = gp.tile([BS, 1], F32)
                nc.scalar.activation(exps[:], dd[:], ACT.Exp, bias=negm[:, 0:1],
                                     accum_out=sumexp[:])
                rsum = gp.tile([BS, 1], F32)
                nc.vector.reciprocal(rsum[:], sumexp[:])
                sc = gp.tile([BS, E], F32)
                nc.vector.tensor_scalar(sc[:], exps[:], rsum[:, 0:1], None, OP.mult)
                nc.sync.dma_start(scores_d[:], sc[:])

                # top-2 via DVE top-8 sort
                mx8 = gp.tile([BS, 8], F32)
                nc.vector.max(mx8[:], sc[:])
                ix8 = gp.tile([BS, 8], mybir.dt.uint32)
                nc.vector.max_index(ix8[:], mx8[:], sc[:])
                idxf = gp.tile([BS, 2], F32)
                nc.vector.tensor_copy(idxf[:], ix8[:, 0:2])
                idxi = gp.tile([BS, 2], I32)
                nc.vector.tensor_copy(idxi[:], ix8[:, 0:2])
                nc.sync.dma_start(idxs_d[:], idxi[:])
                msel = gp.tile([BS, 2], F32)
                nc.vector.tensor_copy(msel[:], mx8[:, 0:2])
                e128 = gp.tile([BS, 2], F32)
                nc.vector.tensor_scalar(e128[:], idxf[:], 128.0, None, OP.mult)

                # pack [8,2]+[8,2] to one partition, broadcast to 128 partitions
                bsrc = gp.tile([1, 32], F32)
                nc.sync.dma_start(
                    bsrc[0:1, 0:16].rearrange("p (b i) -> p b i", b=BS), e128[:])
                nc.sync.dma_start(
                    bsrc[0:1, 16:32].rearrange("p (b i) -> p b i", b=BS), msel[:])
                bc_ps = gps2.tile([128, 32], F32, tag="bc")
                nc.tensor.matmul(bc_ps[:], ones1_t[0:1, :], bsrc[0:1, :],
                                 start=True, stop=True)
                msb = cp.tile([128, 16], F32)
                nc.vector.tensor_copy(msb[:], bc_ps[:, 16:32])
                e128b = gp.tile([128, 16], F32)
                nc.vector.tensor_copy(e128b[:], bc_ps[:, 0:16])

                gidxf = gp.tile([128, 16, 8], F32)
                for si in range(16):
                    nc.vector.tensor_scalar(gidxf[:, si, :], base16_t[:],
                                            e128b[:, si:si + 1], None, OP.add)
                gidx = cp.tile([128, 16 * 8], I16)
                nc.vector.tensor_copy(gidx[:], gidxf[:].rearrange("p a b -> p (a b)"))

            # ---- main expert-conv loop ----
            with tc.tile_pool(name="wp", bufs=2) as wp, \
                 tc.tile_pool(name="nfp", bufs=3) as nfp, \
                 tc.tile_pool(name="eop", bufs=6) as eop, \
                 tc.tile_pool(name="accp", bufs=2) as accp, \
                 tc.tile_pool(name="cps", bufs=2, space="PSUM") as cps, \
                 tc.tile_pool(name="cps1", bufs=2, space="PSUM") as cps1:
                GROUPS = [(0, 1), (1, 1), (2, 2), (4, 4), (8, 4), (12, 4)]
                slot_map = {}
                wgroups = []
                for g, (st, sz) in enumerate(GROUPS):
                    wg = wp.tile([128, 4, WROW], BF16, tag="w")
                    nc.gpsimd.dma_gather(wg[:, 0:sz, :], wtab[:],
                                         gidx[:, st * 8:(st + sz) * 8],
                                         sz * 128, sz * 128, WROW)
                    wgroups.append(wg)
                    for k in range(sz):
                        slot_map[st + k] = (g, k)
                for b in range(BS):
                    acc = accp.tile([128, 2, 2, HALF], F32, tag="acc")
                    for i in range(2):
                        si = b * 2 + i
                        g, k = slot_map[si]
                        wsb = wgroups[g][:, k:k + 1, :]
                        nf = nfp.tile([128, 2, HALF], BF16, tag="nf")
                        ps3 = cps.tile([128, 2, 512], F32, tag="ps3")
                        j = 0
                        for dy in range(3):
                            for dx in range(3):
                                for cb in range(2):
                                    w_ap = wsb[:, 0, j * 128:(j + 1) * 128]
                                    for half in range(2):
                                        rhs = xviewb(b, cb, half * 14 + dy, dx, 14)
                                        nc.tensor.matmul(
                                            ps3[:, half, 0:HALF], w_ap,
                                            rhs, start=(j == 0), stop=(j == 17))
                                    j += 1
                        for half in range(2):
                            nc.vector.tensor_copy(nf[:, half, :],
                                                  ps3[:, half, 0:HALF])

                        for ob in range(2):
                            ps1 = cps1.tile([128, 2, 512], F32, tag="ps1")
                            for ib in range(3):
                                w0 = W3_COLS + (ib * 2 + ob) * 128
                                w_ap = wsb[:, 0, w0:w0 + 128]
                                for half in range(2):
                                    if ib < 2:
                                        rhs = xviewb(b, ib, 1 + half * 14, 1, 14)
                                    else:
                                        rhs = nf[:, half, :]
                                    nc.tensor.matmul(
                                        ps1[:, half, 0:HALF], w_ap, rhs,
                                        start=(ib == 0), stop=(ib == 2))
                            for half in range(2):
                                eo = eop.tile([128, HALF], F32, tag="eo")
                                nc.vector.tensor_copy(eo[:], ps1[:, half, 0:HALF])
                                nc.sync.dma_start(
                                    eo_d[b, i, ob * 128:(ob + 1) * 128,
                                         half * HALF:(half + 1) * HALF], eo[:])
                                if i == 0:
                                    nc.vector.tensor_scalar(
                                        acc[:, ob, half, :], eo[:],
                                        msb[:, si:si + 1], None, OP.mult)
                                else:
                                    tmp = eop.tile([128, HALF], F32, tag="tmp")
                                    nc.vector.tensor_scalar(
                                        tmp[:], eo[:], msb[:, si:si + 1], None,
                                        OP.mult)
                                    nc.vector.tensor_tensor(
                                        acc[:, ob, half, :], acc[:, ob, half, :],
                                        tmp[:], OP.add)
                    for ob in range(2):
                        nc.sync.dma_start(
                            out_d[b, ob * 128:(ob + 1) * 128, :]
                            .rearrange("p (h q) -> p h q", h=2),
                            acc[:, ob, :, :])

    nc.finalize()
    return nc


def _host_prep(csp, ccp, proto):
    w3 = csp.transpose(0, 3, 4, 2, 1).reshape(E, 3, 3, 2, 128, K)
    w3 = w3.transpose(0, 4, 1, 2, 3, 5).reshape(E, 128, W3_COLS)
    w1 = ccp[:, :, :, 0, 0].reshape(E, 2, 128, 3, 128)
    w1 = w1.transpose(0, 4, 3, 1, 2).reshape(E, 128, W1_COLS)
    import ml_dtypes
    wtab = np.concatenate([w3, w1], axis=2).reshape(E * 128, WROW)
    wtab = np.ascontiguousarray(wtab.astype(ml_dtypes.bfloat16))

    protoT = np.zeros((128, 7, E), np.float32)
    for j in range(7):
        seg = proto[:, j * 128:min((j + 1) * 128, HW)]
        protoT[:seg.shape[1], j, :] = seg.T
    protoT = protoT.reshape(128, 7 * E)

    pn2 = np.broadcast_to((proto.astype(np.float32) ** 2).sum(axis=1)[None, :],
                          (BS, E)).copy()
    base16 = (np.arange(8)[None, :] * 16 + (np.arange(128) % 16)[:, None]).astype(
        np.float32)
    iota16 = np.broadcast_to(np.arange(E, dtype=np.float32)[None, :], (BS, E)).copy()
    iota1e9 = iota16 + 1e4
    ident = np.eye(128, dtype=np.float32)
    ones1 = np.ones((1, 128), np.float32)
    cmean = np.full((128, 1), 1.0 / C, np.float32)
    return dict(wtab=wtab, protoT=protoT, pn2=pn2, base16=base16, iota16=iota16,
                iota1e9=iota1e9, ident=ident, ones1=ones1, cmean=cmean)


def kernel(pretrained_x, x, conv_special_param, conv_channel_param, prototype,
           topk):
    assert int(topk) == 2
    x = np.asarray(x, np.float32).reshape(B, C, S, S)
    xpad = np.zeros((B, C, SP, SP), np.float32)
    xpad[:, :, 1:1 + S, 1:1 + S] = x
    xpad = xpad.reshape(B, C, SP * SP)

    consts = _host_prep(np.asarray(conv_special_param, np.float32),
                        np.asarray(conv_channel_param, np.float32),
                        np.asarray(prototype, np.float32))

    if "nc" not in _cache:
        _cache["nc"] = _build_program()
    nc = _cache["nc"]

    in_maps = []
    for c in range(NCORES):
        m = dict(consts)
        m["x_in"] = xpad[c * BS:(c + 1) * BS]
        in_maps.append(m)

    res = run_bass_kernel_spmd(nc, in_maps, core_ids=list(range(NCORES)),
                               **_cache.get("run_kwargs", {}))
    kernel.last_results = res

    outs = res.results
    out = np.concatenate([r["out_d"] for r in outs]).reshape(B, C, S, S)
    scores = np.concatenate([r["scores_d"] for r in outs])
    idxs = np.concatenate([r["idxs_d"] for r in outs]).astype(np.int32)
    query_fm = np.concatenate([r["qf_d"] for r in outs])
    experts_out = np.concatenate([r["eo_d"] for r in outs]).reshape(B, 2, C, S, S)
    return out, scores, idxs, query_fm, experts_out


# revision 20
# speedup vs baseline: 1.0742x; 1.0398x over previous
"""Trainium2 Bass kernel for nn_CNN_MoE_v1 (moe_routing).

Strategy: data-parallel over batch across 8 NeuronCores (8 samples/core).
Per core, on device:
  - gating (fp32): channel-mean -> normalized prototype distances ->
    softmax -> top-2 (value + index, lax.top_k tie semantics)
  - expert weights fetched per sample-slot with dma_gather (indices
    computed on device), weights pre-laid-out on host as one table
    row-per-(expert, contract-partition)
  - 3x3 conv as 18 accumulated fp32r matmuls (9 shifts x 2 c-blocks) per
    output half, 1x1 conv as 3x2 fp32r matmuls reading x + new features
  - experts_out written per (slot, o-block, half); out = sum of
    score-weighted expert outputs (DVE)
All five reference outputs are produced: (out, scores, idxs, query_fm,
experts_out).
"""

import sys

for _p in ('/opt/trn_rl_repo', '/root/.axon_site/_ro/trn_rl_repo'):
    if _p not in sys.path:
        try:
            import concourse  # noqa: F401
            break
        except Exception:
            sys.path.insert(0, _p)

import numpy as np

import concourse.bass as bass
import concourse.mybir as mybir
import concourse.tile as tile
from concourse import bacc
from concourse.bass_utils import run_bass_kernel_spmd

F32 = mybir.dt.float32
F32R = mybir.dt.float32r
BF16 = mybir.dt.bfloat16
FP16 = mybir.dt.float16
I32 = mybir.dt.int32
I16 = mybir.dt.int16
AX = mybir.AxisListType
OP = mybir.AluOpType
ACT = mybir.ActivationFunctionType

B, C, S, K, E = 64, 256, 28, 128, 16
NCORES = 8
BS = B // NCORES          # samples per core
HW = S * S                # 784
SP = S + 2                # 30 (padded)
HALF = HW // 2            # 392
W3_COLS = 9 * 2 * 128     # 2304
W1_COLS = 3 * 2 * 128     # 768
WROW = W3_COLS + W1_COLS  # 3072

_cache = {}


def _build_program():
    nc = bacc.Bacc()

    x_in = nc.declare_dram_parameter("x_in", [BS, C, SP * SP], FP16, isOutput=False)
    wtab = nc.declare_dram_parameter("wtab", [E * 128, WROW], BF16, isOutput=False)
    protoT = nc.declare_dram_parameter("protoT", [128, 7 * E], F32, isOutput=False)
    pn2 = nc.declare_dram_parameter("pn2", [BS, E], F32, isOutput=False)
    base16 = nc.declare_dram_parameter("base16", [128, 8], F32, isOutput=False)
    iota16 = nc.declare_dram_parameter("iota16", [BS, E], F32, isOutput=False)
    iota1e9 = nc.declare_dram_parameter("iota1e9", [BS, E], F32, isOutput=False)
    ident = nc.declare_dram_parameter("ident", [128, 128], F32, isOutput=False)
    ones1 = nc.declare_dram_parameter("ones1", [1, 128], F32, isOutput=False)
    cmean = nc.declare_dram_parameter("cmean", [128, 1], F32R, isOutput=False)

    out_d = nc.declare_dram_parameter("out_d", [BS, C, HW], F32, isOutput=True)
    scores_d = nc.declare_dram_parameter("scores_d", [BS, E], F32, isOutput=True)
    idxs_d = nc.declare_dram_parameter("idxs_d", [BS, 2], I32, isOutput=True)
    qf_d = nc.declare_dram_parameter("qf_d", [BS, HW], F32, isOutput=True)
    eo_d = nc.declare_dram_parameter("eo_d", [BS, 2, C, HW], F32, isOutput=True)

    with tile.TileContext(nc) as tc:
        with tc.tile_pool(name="const", bufs=1) as cp, \
             tc.tile_pool(name="xp", bufs=1) as xp, \
             tc.tile_pool(name="xgp", bufs=8) as xgp, \
             tc.tile_pool(name="xfp", bufs=3) as xfp, \
             tc.tile_pool(name="gate", bufs=1) as gp, \
             tc.tile_pool(name="qtp", bufs=3) as qtp:

            # ---- prefetch sample 0 gating-x before anything else ----
            xg0 = xgp.tile([128, 2, SP * SP], FP16, tag="xg16")
            for cb in range(2):
                nc.sync.dma_start(xg0[:, cb, :],
                                  x_in[0, cb * 128:(cb + 1) * 128, :])
            protoT_t = cp.tile([128, 7, E], F32)
            nc.sync.dma_start(protoT_t[:], protoT[:].rearrange("p (j e) -> p j e", j=7))
            pn2_t = cp.tile([BS, E], F32)
            nc.sync.dma_start(pn2_t[:], pn2[:])
            base16_t = cp.tile([128, 8], F32)
            nc.sync.dma_start(base16_t[:], base16[:])
            iota16_t = cp.tile([BS, E], F32)
            nc.sync.dma_start(iota16_t[:], iota16[:])
            iota1e9_t = cp.tile([BS, E], F32)
            nc.sync.dma_start(iota1e9_t[:], iota1e9[:])
            ident_t = cp.tile([128, 128], F32)
            nc.sync.dma_start(ident_t[:], ident[:])
            ones1_t = cp.tile([1, 128], F32)
            nc.sync.dma_start(ones1_t[:], ones1[:])
            cmean_t = cp.tile([128, 1], F32R)
            nc.sync.dma_start(cmean_t[:], cmean[:])

            # ---- x load into zero-padded [128, b, cb, 30*30] ----
            xbf = xp.tile([128, BS, 2, SP * SP], BF16)

            def xviewb(b, cb, row0, col0, nrows):
                """[128, nrows, 28] strided bf16 view (convs)."""
                return xbf[:, b, cb, :].rearrange("p (r c) -> p r c", r=SP)[
                    :, row0:row0 + nrows, col0:col0 + S]

            # ---- gating ----
            qf8 = gp.tile([BS, 896], F32)
            _ = None
            nc.vector.memset(qf8[:, 784:], 0.0)

            with tc.tile_pool(name="gps", bufs=2, space="PSUM") as gps, \
                 tc.tile_pool(name="gps2", bufs=1, space="PSUM") as gps2:
                for b in range(BS):
                    if b == 0:
                        xg16 = xg0
                    else:
                        xg16 = xgp.tile([128, 2, SP * SP], FP16, tag="xg16")
                        for cb in range(2):
                            nc.sync.dma_start(
                                xg16[:, cb, :],
                                x_in[b, cb * 128:(cb + 1) * 128, :])
                    xg = xfp.tile([128, 2, SP * SP], F32R, tag="xg")
                    nc.vector.tensor_copy(xg[:], xg16[:])
                    qf_ps = gps.tile([1, 1024], F32, tag="qf")
                    for half in range(2):
                        o0 = 0 if half == 0 else 512
                        out_ap = qf_ps[0:1, o0:o0 + HALF]
                        for cb in range(2):
                            rhs = xg[:, cb, :].rearrange(
                                "p (r c) -> p r c", r=SP)[
                                :, 1 + half * 14:15 + half * 14, 1:1 + S]
                            nc.tensor.matmul(out_ap, cmean_t[:, 0:1], rhs,
                                             start=(cb == 0), stop=(cb == 1))
                    nc.scalar.copy(xbf[:, b, :, :], xg16[:])
                    qftmp = qtp.tile([1, HW], F32, tag="qftmp")
                    nc.vector.tensor_copy(
                        qftmp[0:1, :].rearrange("p (c q) -> p c q", c=2),
                        qf_ps[0:1, :].rearrange("p (c q) -> p c q", c=2)[:, :, 0:HALF])
                    nc.sync.dma_start(qf8[b:b + 1, 0:HW], qftmp[0:1, :])

                nc.sync.dma_start(qf_d[:], qf8[0:BS, 0:HW])

                # qn2 = sum(qf^2); inv = 1/max(sqrt(qn2),1e-12)
                sq = gp.tile([BS, HW], F32)
                qn2 = gp.tile([BS, 1], F32)
                nc.scalar.activation(sq[:], qf8[0:BS, 0:HW], ACT.Square,
                                     accum_out=qn2[:])
                nrm = gp.tile([BS, 1], F32)
                nc.scalar.sqrt(nrm[:], qn2[:])
                nrmc = gp.tile([BS, 1], F32)
                nc.vector.tensor_scalar_max(nrmc[:], nrm[:], 1e-12)
                inv = gp.tile([BS, 1], F32)
                nc.vector.reciprocal(inv[:], nrmc[:])
                qn2i = gp.tile([BS, 1], F32)
                nc.vector.tensor_scalar(qn2i[:], qn2[:], inv[:, 0:1], inv[:, 0:1],
                                        OP.mult, OP.mult)

                # qfT via PE transpose: [8, 128] -> [128, 8] per 128-chunk
                qfT = gp.tile([128, 7, BS], F32)
                for j in range(7):
                    tp = gps.tile([128, BS], F32, tag="tp")
                    nc.tensor.transpose(tp[:], qf8[0:BS, j * 128:(j + 1) * 128],
                                        ident_t[0:BS, 0:BS])
                    nc.vector.tensor_copy(qfT[:, j, :], tp[:])

                # SS[b,e] = qf . proto_e
                ss_ps = gps2.tile([BS, E], F32, tag="ss")
                for j in range(7):
                    nc.tensor.matmul(ss_ps[:], qfT[:, j, :], protoT_t[:, j, :],
                                     start=(j == 0), stop=(j == 6))

                # d2 = qn2*inv^2 + pn2 - 2*SS*inv ; d = sqrt(max(d2,0))
                t1 = gp.tile([BS, E], F32)
                nc.vector.tensor_scalar(t1[:], ss_ps[:], inv[:, 0:1], -2.0,
                                        OP.mult, OP.mult)
                t2 = gp.tile([BS, E], F32)
                nc.vector.tensor_scalar(t2[:], t1[:], qn2i[:, 0:1], None, OP.add)
                d2 = gp.tile([BS, E], F32)
                nc.vector.tensor_tensor(d2[:], t2[:], pn2_t[:], OP.add)
                d2c = gp.tile([BS, E], F32)
                nc.vector.tensor_scalar_max(d2c[:], d2[:], 0.0)
                dd = gp.tile([BS, E], F32)
                nc.scalar.sqrt(dd[:], d2c[:])

                # softmax over 16
                mx = gp.tile([BS, 1], F32)
                nc.vector.tensor_reduce(mx[:], dd[:], axis=AX.X, op=OP.max)
                negm = gp.tile([BS, 1], F32)
                nc.vector.tensor_scalar_mul(negm[:], mx[:], -1.0)
                exps = gp.tile([BS, E], F32)
                sumexp = gp.tile([BS, 1], F32)
                nc.scalar.activation(exps[:], dd[:], ACT.Exp, bias=negm[:, 0:1],
                                     accum_out=sumexp[:])
                rsum = gp.tile([BS, 1], F32)
                nc.vector.reciprocal(rsum[:], sumexp[:])
                sc = gp.tile([BS, E], F32)
                nc.vector.tensor_scalar(sc[:], exps[:], rsum[:, 0:1], None, OP.mult)
                nc.sync.dma_start(scores_d[:], sc[:])

                # top-2 via DVE top-8 sort
                mx8 = gp.tile([BS, 8], F32)
                nc.vector.max(mx8[:], sc[:])
                ix8 = gp.tile([BS, 8], mybir.dt.uint32)
                nc.vector.max_index(ix8[:], mx8[:], sc[:])
                idxf = gp.tile([BS, 2], F32)
                nc.vector.tensor_copy(idxf[:], ix8[:, 0:2])
                idxi = gp.tile([BS, 2], I32)
                nc.vector.tensor_copy(idxi[:], ix8[:, 0:2])
                nc.sync.dma_start(idxs_d[:], idxi[:])
                msel = gp.tile([BS, 2], F32)
                nc.vector.tensor_copy(msel[:], mx8[:, 0:2])
                e128 = gp.tile([BS, 2], F32)
                nc.vector.tensor_scalar(e128[:], idxf[:], 128.0, None, OP.mult)

                # pack [8,2]+[8,2] to one partition, broadcast to 128 partitions
                bsrc = gp.tile([1, 32], F32)
                nc.sync.dma_start(
                    bsrc[0:1, 0:16].rearrange("p (b i) -> p b i", b=BS), e128[:])
                nc.sync.dma_start(
                    bsrc[0:1, 16:32].rearrange("p (b i) -> p b i", b=BS), msel[:])
                bc_ps = gps2.tile([128, 32], F32, tag="bc")
                nc.tensor.matmul(bc_ps[:], ones1_t[0:1, :], bsrc[0:1, :],
                                 start=True, stop=True)
                msb = cp.tile([128, 16], F32)
                nc.vector.tensor_copy(msb[:], bc_ps[:, 16:32])
                e128b = gp.tile([128, 16], F32)
                nc.vector.tensor_copy(e128b[:], bc_ps[:, 0:16])

                gidxf = gp.tile([128, 16, 8], F32)
                for si in range(16):
                    nc.vector.tensor_scalar(gidxf[:, si, :], base16_t[:],
                                            e128b[:, si:si + 1], None, OP.add)
                gidx = cp.tile([128, 16 * 8], I16)
                nc.vector.tensor_copy(gidx[:], gidxf[:].rearrange("p a b -> p (a b)"))

            # ---- main expert-conv loop ----
            with tc.tile_pool(name="wp", bufs=2) as wp, \
                 tc.tile_pool(name="nfp", bufs=3) as nfp, \
                 tc.tile_pool(name="eop", bufs=6) as eop, \
                 tc.tile_pool(name="accp", bufs=2) as accp, \
                 tc.tile_pool(name="cps", bufs=2, space="PSUM") as cps, \
                 tc.tile_pool(name="cps1", bufs=2, space="PSUM") as cps1:
                GROUPS = [(0, 1), (1, 1), (2, 2), (4, 4), (8, 4), (12, 4)]
                slot_map = {}
                wgroups = []
                for g, (st, sz) in enumerate(GROUPS):
                    wg = wp.tile([128, 4, WROW], BF16, tag="w")
                    nc.gpsimd.dma_gather(wg[:, 0:sz, :], wtab[:],
                                         gidx[:, st * 8:(st + sz) * 8],
                                         sz * 128, sz * 128, WROW)
                    wgroups.append(wg)
                    for k in range(sz):
                        slot_map[st + k] = (g, k)
                for b in range(BS):
                    acc = accp.tile([128, 2, 2, HALF], F32, tag="acc")
                    for i in range(2):
                        si = b * 2 + i
                        g, k = slot_map[si]
                        wsb = wgroups[g][:, k:k + 1, :]
                        nf = nfp.tile([128, 2, HALF], BF16, tag="nf")
                        ps3 = cps.tile([128, 2, 512], F32, tag="ps3")
                        j = 0
                        for dy in range(3):
                            for dx in range(3):
                                for cb in range(2):
                                    w_ap = wsb[:, 0, j * 128:(j + 1) * 128]
                                    for half in range(2):
                                        rhs = xviewb(b, cb, half * 14 + dy, dx, 14)
                                        nc.tensor.matmul(
                                            ps3[:, half, 0:HALF], w_ap,
                                            rhs, start=(j == 0), stop=(j == 17))
                                    j += 1
                        for half in range(2):
                            nc.vector.tensor_copy(nf[:, half, :],
                                                  ps3[:, half, 0:HALF])

                        for ob in range(2):
                            ps1 = cps1.tile([128, 2, 512], F32, tag="ps1")
                            for ib in range(3):
                                w0 = W3_COLS + (ib * 2 + ob) * 128
                                w_ap = wsb[:, 0, w0:w0 + 128]
                                for half in range(2):
                                    if ib < 2:
                                        rhs = xviewb(b, ib, 1 + half * 14, 1, 14)
                                    else:
                                        rhs = nf[:, half, :]
                                    nc.tensor.matmul(
                                        ps1[:, half, 0:HALF], w_ap, rhs,
                                        start=(ib == 0), stop=(ib == 2))
                            for half in range(2):
                                eo = eop.tile([128, HALF], F32, tag="eo")
                                nc.vector.tensor_copy(eo[:], ps1[:, half, 0:HALF])
                                nc.sync.dma_start(
                                    eo_d[b, i, ob * 128:(ob + 1) * 128,
                                         half * HALF:(half + 1) * HALF], eo[:])
                                if i == 0:
                                    nc.vector.tensor_scalar(
                                        acc[:, ob, half, :], eo[:],
                                        msb[:, si:si + 1], None, OP.mult)
                                else:
                                    tmp = eop.tile([128, HALF], F32, tag="tmp")
                                    nc.vector.tensor_scalar(
                                        tmp[:], eo[:], msb[:, si:si + 1], None,
                                        OP.mult)
                                    nc.vector.tensor_tensor(
                                        acc[:, ob, half, :], acc[:, ob, half, :],
                                        tmp[:], OP.add)
                    for ob in range(2):
                        nc.sync.dma_start(
                            out_d[b, ob * 128:(ob + 1) * 128, :]
                            .rearrange("p (h q) -> p h q", h=2),
                            acc[:, ob, :, :])

    nc.finalize()
    return nc


def _host_prep(csp, ccp, proto):
    w3 = csp.transpose(0, 3, 4, 2, 1).reshape(E, 3, 3, 2, 128, K)
    w3 = w3.transpose(0, 4, 1, 2, 3, 5).reshape(E, 128, W3_COLS)
    w1 = ccp[:, :, :, 0, 0].reshape(E, 2, 128, 3, 128)
    w1 = w1.transpose(0, 4, 3, 1, 2).reshape(E, 128, W1_COLS)
    import ml_dtypes
    wtab = np.concatenate([w3, w1], axis=2).reshape(E * 128, WROW)
    wtab = np.ascontiguousarray(wtab.astype(ml_dtypes.bfloat16))

    protoT = np.zeros((128, 7, E), np.float32)
    for j in range(7):
        seg = proto[:, j * 128:min((j + 1) * 128, HW)]
        protoT[:seg.shape[1], j, :] = seg.T
    protoT = protoT.reshape(128, 7 * E)

    pn2 = np.broadcast_to((proto.astype(np.float32) ** 2).sum(axis=1)[None, :],
                          (BS, E)).copy()
    base16 = (np.arange(8)[None, :] * 16 + (np.arange(128) % 16)[:, None]).astype(
        np.float32)
    iota16 = np.broadcast_to(np.arange(E, dtype=np.float32)[None, :], (BS, E)).copy()
    iota1e9 = iota16 + 1e4
    ident = np.eye(128, dtype=np.float32)
    ones1 = np.ones((1, 128), np.float32)
    cmean = np.full((128, 1), 1.0 / C, np.float32)
    return dict(wtab=wtab, protoT=protoT, pn2=pn2, base16=base16, iota16=iota16,
                iota1e9=iota1e9, ident=ident, ones1=ones1, cmean=cmean)


def kernel(pretrained_x, x, conv_special_param, conv_channel_param, prototype,
           topk):
    assert int(topk) == 2
    x = np.asarray(x, np.float32).reshape(B, C, S, S)
    xpad = np.zeros((B, C, SP, SP), np.float32)
    xpad[:, :, 1:1 + S, 1:1 + S] = x
    xpad = xpad.reshape(B, C, SP * SP)

    consts = _host_prep(np.asarray(conv_special_param, np.float32),
                        np.asarray(conv_channel_param, np.float32),
                        np.asarray(prototype, np.float32))

    if "nc" not in _cache:
        _cache["nc"] = _build_program()
    nc = _cache["nc"]

    in_maps = []
    for c in range(NCORES):
        m = dict(consts)
        m["x_in"] = xpad.astype(np.float16)[c * BS:(c + 1) * BS]
        in_maps.append(m)

    res = run_bass_kernel_spmd(nc, in_maps, core_ids=list(range(NCORES)),
                               **_cache.get("run_kwargs", {}))
    kernel.last_results = res

    outs = res.results
    out = np.concatenate([r["out_d"] for r in outs]).reshape(B, C, S, S)
    scores = np.concatenate([r["scores_d"] for r in outs])
    idxs = np.concatenate([r["idxs_d"] for r in outs]).astype(np.int32)
    query_fm = np.concatenate([r["qf_d"] for r in outs])
    experts_out = np.concatenate([r["eo_d"] for r in outs]).reshape(B, 2, C, S, S)
    return out, scores, idxs, query_fm, experts_out


# revision 22
# speedup vs baseline: 1.1161x; 1.0390x over previous
"""Trainium2 Bass kernel for nn_CNN_MoE_v1 (moe_routing).

Strategy: data-parallel over batch across 8 NeuronCores (8 samples/core).
Per core, on device:
  - gating (fp32): channel-mean -> normalized prototype distances ->
    softmax -> top-2 (value + index, lax.top_k tie semantics)
  - expert weights fetched per sample-slot with dma_gather (indices
    computed on device), weights pre-laid-out on host as one table
    row-per-(expert, contract-partition)
  - 3x3 conv as 18 accumulated fp32r matmuls (9 shifts x 2 c-blocks) per
    output half, 1x1 conv as 3x2 fp32r matmuls reading x + new features
  - experts_out written per (slot, o-block, half); out = sum of
    score-weighted expert outputs (DVE)
All five reference outputs are produced: (out, scores, idxs, query_fm,
experts_out).
"""

import sys

for _p in ('/opt/trn_rl_repo', '/root/.axon_site/_ro/trn_rl_repo'):
    if _p not in sys.path:
        try:
            import concourse  # noqa: F401
            break
        except Exception:
            sys.path.insert(0, _p)

import numpy as np

import concourse.bass as bass
import concourse.mybir as mybir
import concourse.tile as tile
from concourse import bacc
from concourse.bass_utils import run_bass_kernel_spmd

F32 = mybir.dt.float32
F32R = mybir.dt.float32r
BF16 = mybir.dt.bfloat16
FP16 = mybir.dt.float16
I32 = mybir.dt.int32
I16 = mybir.dt.int16
AX = mybir.AxisListType
OP = mybir.AluOpType
ACT = mybir.ActivationFunctionType

B, C, S, K, E = 64, 256, 28, 128, 16
NCORES = 8
BS = B // NCORES          # samples per core
HW = S * S                # 784
SP = S + 2                # 30 (padded)
HALF = HW // 2            # 392
W3_COLS = 9 * 2 * 128     # 2304
W1_COLS = 3 * 2 * 128     # 768
WROW = W3_COLS + W1_COLS  # 3072

_cache = {}


def _build_program():
    nc = bacc.Bacc()

    x_in = nc.declare_dram_parameter("x_in", [BS, C, SP * SP], FP16, isOutput=False)
    wtab = nc.declare_dram_parameter("wtab", [E * 128, WROW], BF16, isOutput=False)
    protoT = nc.declare_dram_parameter("protoT", [128, 7 * E], F32, isOutput=False)
    pn2 = nc.declare_dram_parameter("pn2", [BS, E], F32, isOutput=False)
    base16 = nc.declare_dram_parameter("base16", [128, 8], F32, isOutput=False)
    iota16 = nc.declare_dram_parameter("iota16", [BS, E], F32, isOutput=False)
    iota1e9 = nc.declare_dram_parameter("iota1e9", [BS, E], F32, isOutput=False)
    ident = nc.declare_dram_parameter("ident", [128, 128], F32, isOutput=False)
    ones1 = nc.declare_dram_parameter("ones1", [1, 128], F32, isOutput=False)
    cmean = nc.declare_dram_parameter("cmean", [128, 1], F32R, isOutput=False)

    out_d = nc.declare_dram_parameter("out_d", [BS, C, HW], F32, isOutput=True)
    scores_d = nc.declare_dram_parameter("scores_d", [BS, E], F32, isOutput=True)
    idxs_d = nc.declare_dram_parameter("idxs_d", [BS, 2], I32, isOutput=True)
    qf_d = nc.declare_dram_parameter("qf_d", [BS, HW], F32, isOutput=True)
    eo_d = nc.declare_dram_parameter("eo_d", [BS, 2, C, HW], F32, isOutput=True)

    with tile.TileContext(nc) as tc:
        with tc.tile_pool(name="const", bufs=1) as cp, \
             tc.tile_pool(name="xp", bufs=1) as xp, \
             tc.tile_pool(name="xgp", bufs=6) as xgp, \
             tc.tile_pool(name="xfp", bufs=3) as xfp, \
             tc.tile_pool(name="gate", bufs=1) as gp, \
             tc.tile_pool(name="qtp", bufs=3) as qtp:

            # ---- prefetch sample 0 gating-x before anything else ----
            xg0 = xgp.tile([128, 2, SP * SP], FP16, tag="xg16")
            for cb in range(2):
                nc.sync.dma_start(xg0[:, cb, :],
                                  x_in[0, cb * 128:(cb + 1) * 128, :])
            protoT_t = cp.tile([128, 7, E], F32)
            nc.sync.dma_start(protoT_t[:], protoT[:].rearrange("p (j e) -> p j e", j=7))
            pn2_t = cp.tile([BS, E], F32)
            nc.sync.dma_start(pn2_t[:], pn2[:])
            base16_t = cp.tile([128, 8], F32)
            nc.sync.dma_start(base16_t[:], base16[:])
            iota16_t = cp.tile([BS, E], F32)
            nc.sync.dma_start(iota16_t[:], iota16[:])
            iota1e9_t = cp.tile([BS, E], F32)
            nc.sync.dma_start(iota1e9_t[:], iota1e9[:])
            ident_t = cp.tile([128, 128], F32)
            nc.sync.dma_start(ident_t[:], ident[:])
            ones1_t = cp.tile([1, 128], F32)
            nc.sync.dma_start(ones1_t[:], ones1[:])
            cmean_t = cp.tile([128, 1], F32R)
            nc.sync.dma_start(cmean_t[:], cmean[:])

            # ---- x load into zero-padded [128, b, cb, 30*30] ----
            xbf = xp.tile([128, BS, 2, SP * SP], BF16)

            def xviewb(b, cb, row0, col0, nrows):
                """[128, nrows, 28] strided bf16 view (convs)."""
                return xbf[:, b, cb, :].rearrange("p (r c) -> p r c", r=SP)[
                    :, row0:row0 + nrows, col0:col0 + S]

            # ---- gating ----
            qf8 = gp.tile([BS, 896], F32)
            _ = None
            nc.vector.memset(qf8[:, 784:], 0.0)

            with tc.tile_pool(name="gps", bufs=2, space="PSUM") as gps, \
                 tc.tile_pool(name="gps2", bufs=1, space="PSUM") as gps2:
                for b in range(BS):
                    if b == 0:
                        xg16 = xg0
                    else:
                        xg16 = xgp.tile([128, 2, SP * SP], FP16, tag="xg16")
                        for cb in range(2):
                            nc.sync.dma_start(
                                xg16[:, cb, :],
                                x_in[b, cb * 128:(cb + 1) * 128, :])
                    xg = xfp.tile([128, 2, SP * SP], F32R, tag="xg")
                    nc.vector.tensor_copy(xg[:], xg16[:])
                    qf_ps = gps.tile([1, 1024], F32, tag="qf")
                    for half in range(2):
                        o0 = 0 if half == 0 else 512
                        out_ap = qf_ps[0:1, o0:o0 + HALF]
                        for cb in range(2):
                            rhs = xg[:, cb, :].rearrange(
                                "p (r c) -> p r c", r=SP)[
                                :, 1 + half * 14:15 + half * 14, 1:1 + S]
                            nc.tensor.matmul(out_ap, cmean_t[:, 0:1], rhs,
                                             start=(cb == 0), stop=(cb == 1))
                    nc.scalar.copy(xbf[:, b, :, :], xg16[:])
                    qftmp = qtp.tile([1, HW], F32, tag="qftmp")
                    nc.vector.tensor_copy(
                        qftmp[0:1, :].rearrange("p (c q) -> p c q", c=2),
                        qf_ps[0:1, :].rearrange("p (c q) -> p c q", c=2)[:, :, 0:HALF])
                    nc.sync.dma_start(qf8[b:b + 1, 0:HW], qftmp[0:1, :])

                nc.sync.dma_start(qf_d[:], qf8[0:BS, 0:HW])

                # qn2 = sum(qf^2); inv = 1/max(sqrt(qn2),1e-12)
                sq = gp.tile([BS, HW], F32)
                qn2 = gp.tile([BS, 1], F32)
                nc.scalar.activation(sq[:], qf8[0:BS, 0:HW], ACT.Square,
                                     accum_out=qn2[:])
                nrm = gp.tile([BS, 1], F32)
                nc.scalar.sqrt(nrm[:], qn2[:])
                nrmc = gp.tile([BS, 1], F32)
                nc.vector.tensor_scalar_max(nrmc[:], nrm[:], 1e-12)
                inv = gp.tile([BS, 1], F32)
                nc.vector.reciprocal(inv[:], nrmc[:])
                qn2i = gp.tile([BS, 1], F32)
                nc.vector.tensor_scalar(qn2i[:], qn2[:], inv[:, 0:1], inv[:, 0:1],
                                        OP.mult, OP.mult)

                # qfT via PE transpose: [8, 128] -> [128, 8] per 128-chunk
                qfT = gp.tile([128, 7, BS], F32)
                for j in range(7):
                    tp = gps.tile([128, BS], F32, tag="tp")
                    nc.tensor.transpose(tp[:], qf8[0:BS, j * 128:(j + 1) * 128],
                                        ident_t[0:BS, 0:BS])
                    nc.vector.tensor_copy(qfT[:, j, :], tp[:])

                # SS[b,e] = qf . proto_e
                ss_ps = gps2.tile([BS, E], F32, tag="ss")
                for j in range(7):
                    nc.tensor.matmul(ss_ps[:], qfT[:, j, :], protoT_t[:, j, :],
                                     start=(j == 0), stop=(j == 6))

                # d2 = qn2*inv^2 + pn2 - 2*SS*inv ; d = sqrt(max(d2,0))
                t1 = gp.tile([BS, E], F32)
                nc.vector.tensor_scalar(t1[:], ss_ps[:], inv[:, 0:1], -2.0,
                                        OP.mult, OP.mult)
                t2 = gp.tile([BS, E], F32)
                nc.vector.tensor_scalar(t2[:], t1[:], qn2i[:, 0:1], None, OP.add)
                d2 = gp.tile([BS, E], F32)
                nc.vector.tensor_tensor(d2[:], t2[:], pn2_t[:], OP.add)
                d2c = gp.tile([BS, E], F32)
                nc.vector.tensor_scalar_max(d2c[:], d2[:], 0.0)
                dd = gp.tile([BS, E], F32)
                nc.scalar.sqrt(dd[:], d2c[:])

                # softmax over 16
                mx = gp.tile([BS, 1], F32)
                nc.vector.tensor_reduce(mx[:], dd[:], axis=AX.X, op=OP.max)
                negm = gp.tile([BS, 1], F32)
                nc.vector.tensor_scalar_mul(negm[:], mx[:], -1.0)
                exps = gp.tile([BS, E], F32)
                sumexp = gp.tile([BS, 1], F32)
                nc.scalar.activation(exps[:], dd[:], ACT.Exp, bias=negm[:, 0:1],
                                     accum_out=sumexp[:])
                rsum = gp.tile([BS, 1], F32)
                nc.vector.reciprocal(rsum[:], sumexp[:])
                sc = gp.tile([BS, E], F32)
                nc.vector.tensor_scalar(sc[:], exps[:], rsum[:, 0:1], None, OP.mult)
                nc.sync.dma_start(scores_d[:], sc[:])

                # top-2 via DVE top-8 sort
                mx8 = gp.tile([BS, 8], F32)
                nc.vector.max(mx8[:], sc[:])
                ix8 = gp.tile([BS, 8], mybir.dt.uint32)
                nc.vector.max_index(ix8[:], mx8[:], sc[:])
                idxf = gp.tile([BS, 2], F32)
                nc.vector.tensor_copy(idxf[:], ix8[:, 0:2])
                idxi = gp.tile([BS, 2], I32)
                nc.vector.tensor_copy(idxi[:], ix8[:, 0:2])
                nc.sync.dma_start(idxs_d[:], idxi[:])
                msel = gp.tile([BS, 2], F32)
                nc.vector.tensor_copy(msel[:], mx8[:, 0:2])
                e128 = gp.tile([BS, 2], F32)
                nc.vector.tensor_scalar(e128[:], idxf[:], 128.0, None, OP.mult)

                # pack [8,2]+[8,2] to one partition, broadcast to 128 partitions
                bsrc = gp.tile([1, 32], F32)
                nc.sync.dma_start(
                    bsrc[0:1, 0:16].rearrange("p (b i) -> p b i", b=BS), e128[:])
                nc.sync.dma_start(
                    bsrc[0:1, 16:32].rearrange("p (b i) -> p b i", b=BS), msel[:])
                bc_ps = gps2.tile([128, 32], F32, tag="bc")
                nc.tensor.matmul(bc_ps[:], ones1_t[0:1, :], bsrc[0:1, :],
                                 start=True, stop=True)
                msb = cp.tile([128, 16], F32)
                nc.vector.tensor_copy(msb[:], bc_ps[:, 16:32])
                e128b = gp.tile([128, 16], F32)
                nc.vector.tensor_copy(e128b[:], bc_ps[:, 0:16])

                gidxf = gp.tile([128, 16, 8], F32)
                gidx = cp.tile([128, 16 * 8], I16)
                nc.vector.tensor_scalar(gidxf[:, 0, :], base16_t[:],
                                        e128b[:, 0:1], None, OP.add)
                nc.vector.tensor_copy(gidx[:, 0:8],
                                      gidxf[:, 0, :])
                for si in range(1, 16):
                    nc.vector.tensor_scalar(gidxf[:, si, :], base16_t[:],
                                            e128b[:, si:si + 1], None, OP.add)
                nc.vector.tensor_copy(gidx[:, 8:],
                                      gidxf[:, 1:, :].rearrange("p a b -> p (a b)"))

            # ---- main expert-conv loop ----
            with tc.tile_pool(name="wp", bufs=3) as wp, \
                 tc.tile_pool(name="nfp", bufs=3) as nfp, \
                 tc.tile_pool(name="eop", bufs=6) as eop, \
                 tc.tile_pool(name="accp", bufs=2) as accp, \
                 tc.tile_pool(name="cps", bufs=2, space="PSUM") as cps, \
                 tc.tile_pool(name="cps1", bufs=2, space="PSUM") as cps1:
                GROUPS = [(0, 1), (1, 1), (2, 2), (4, 4), (8, 4), (12, 4)]
                slot_map = {}
                wgroups = []
                for g, (st, sz) in enumerate(GROUPS):
                    wg = wp.tile([128, 4, WROW], BF16, tag="w")
                    nc.gpsimd.dma_gather(wg[:, 0:sz, :], wtab[:],
                                         gidx[:, st * 8:(st + sz) * 8],
                                         sz * 128, sz * 128, WROW)
                    wgroups.append(wg)
                    for k in range(sz):
                        slot_map[st + k] = (g, k)
                for b in range(BS):
                    acc = accp.tile([128, 2, 2, HALF], F32, tag="acc")
                    for i in range(2):
                        si = b * 2 + i
                        g, k = slot_map[si]
                        wsb = wgroups[g][:, k:k + 1, :]
                        nf = nfp.tile([128, 2, HALF], BF16, tag="nf")
                        ps3 = cps.tile([128, 2, 512], F32, tag="ps3")
                        j = 0
                        for dy in range(3):
                            for dx in range(3):
                                for cb in range(2):
                                    w_ap = wsb[:, 0, j * 128:(j + 1) * 128]
                                    for half in range(2):
                                        rhs = xviewb(b, cb, half * 14 + dy, dx, 14)
                                        nc.tensor.matmul(
                                            ps3[:, half, 0:HALF], w_ap,
                                            rhs, start=(j == 0), stop=(j == 17))
                                    j += 1
                        for half in range(2):
                            nc.vector.tensor_copy(nf[:, half, :],
                                                  ps3[:, half, 0:HALF])

                        for ob in range(2):
                            ps1 = cps1.tile([128, 2, 512], F32, tag="ps1")
                            for ib in range(3):
                                w0 = W3_COLS + (ib * 2 + ob) * 128
                                w_ap = wsb[:, 0, w0:w0 + 128]
                                for half in range(2):
                                    if ib < 2:
                                        rhs = xviewb(b, ib, 1 + half * 14, 1, 14)
                                    else:
                                        rhs = nf[:, half, :]
                                    nc.tensor.matmul(
                                        ps1[:, half, 0:HALF], w_ap, rhs,
                                        start=(ib == 0), stop=(ib == 2))
                            for half in range(2):
                                eo = eop.tile([128, HALF], F32, tag="eo")
                                nc.vector.tensor_copy(eo[:], ps1[:, half, 0:HALF])
                                nc.sync.dma_start(
                                    eo_d[b, i, ob * 128:(ob + 1) * 128,
                                         half * HALF:(half + 1) * HALF], eo[:])
                                if i == 0:
                                    nc.vector.tensor_scalar(
                                        acc[:, ob, half, :], eo[:],
                                        msb[:, si:si + 1], None, OP.mult)
                                else:
                                    tmp = eop.tile([128, HALF], F32, tag="tmp")
                                    nc.vector.tensor_scalar(
                                        tmp[:], eo[:], msb[:, si:si + 1], None,
                                        OP.mult)
                                    nc.vector.tensor_tensor(
                                        acc[:, ob, half, :], acc[:, ob, half, :],
                                        tmp[:], OP.add)
                    for ob in range(2):
                        nc.sync.dma_start(
                            out_d[b, ob * 128:(ob + 1) * 128, :]
                            .rearrange("p (h q) -> p h q", h=2),
                            acc[:, ob, :, :])

    nc.finalize()
    return nc


def _host_prep(csp, ccp, proto):
    w3 = csp.transpose(0, 3, 4, 2, 1).reshape(E, 3, 3, 2, 128, K)
    w3 = w3.transpose(0, 4, 1, 2, 3, 5).reshape(E, 128, W3_COLS)
    w1 = ccp[:, :, :, 0, 0].reshape(E, 2, 128, 3, 128)
    w1 = w1.transpose(0, 4, 3, 1, 2).reshape(E, 128, W1_COLS)
    import ml_dtypes
    wtab = np.concatenate([w3, w1], axis=2).reshape(E * 128, WROW)
    wtab = np.ascontiguousarray(wtab.astype(ml_dtypes.bfloat16))

    protoT = np.zeros((128, 7, E), np.float32)
    for j in range(7):
        seg = proto[:, j * 128:min((j + 1) * 128, HW)]
        protoT[:seg.shape[1], j, :] = seg.T
    protoT = protoT.reshape(128, 7 * E)

    pn2 = np.broadcast_to((proto.astype(np.float32) ** 2).sum(axis=1)[None, :],
                          (BS, E)).copy()
    base16 = (np.arange(8)[None, :] * 16 + (np.arange(128) % 16)[:, None]).astype(
        np.float32)
    iota16 = np.broadcast_to(np.arange(E, dtype=np.float32)[None, :], (BS, E)).copy()
    iota1e9 = iota16 + 1e4
    ident = np.eye(128, dtype=np.float32)
    ones1 = np.ones((1, 128), np.float32)
    cmean = np.full((128, 1), 1.0 / C, np.float32)
    return dict(wtab=wtab, protoT=protoT, pn2=pn2, base16=base16, iota16=iota16,
                iota1e9=iota1e9, ident=ident, ones1=ones1, cmean=cmean)


def kernel(pretrained_x, x, conv_special_param, conv_channel_param, prototype,
           topk):
    assert int(topk) == 2
    x = np.asarray(x, np.float32).reshape(B, C, S, S)
    xpad = np.zeros((B, C, SP, SP), np.float32)
    xpad[:, :, 1:1 + S, 1:1 + S] = x
    xpad = xpad.reshape(B, C, SP * SP)

    consts = _host_prep(np.asarray(conv_special_param, np.float32),
                        np.asarray(conv_channel_param, np.float32),
                        np.asarray(prototype, np.float32))

    if "nc" not in _cache:
        _cache["nc"] = _build_program()
    nc = _cache["nc"]

    in_maps = []
    for c in range(NCORES):
        m = dict(consts)
        m["x_in"] = xpad.astype(np.float16)[c * BS:(c + 1) * BS]
        in_maps.append(m)

    res = run_bass_kernel_spmd(nc, in_maps, core_ids=list(range(NCORES)),
                               **_cache.get("run_kwargs", {}))
    kernel.last_results = res

    outs = res.results
    out = np.concatenate([r["out_d"] for r in outs]).reshape(B, C, S, S)
    scores = np.concatenate([r["scores_d"] for r in outs])
    idxs = np.concatenate([r["idxs_d"] for r in outs]).astype(np.int32)
    query_fm = np.concatenate([r["qf_d"] for r in outs])
    experts_out = np.concatenate([r["eo_d"] for r in outs]).reshape(B, 2, C, S, S)
    return out, scores, idxs, query_fm, experts_out


# revision 23
# speedup vs baseline: 1.1341x; 1.0161x over previous
"""Trainium2 Bass kernel for nn_CNN_MoE_v1 (moe_routing).

Strategy: data-parallel over batch across 8 NeuronCores (8 samples/core).
Per core, on device:
  - gating (fp32): channel-mean -> normalized prototype distances ->
    softmax -> top-2 (value + index, lax.top_k tie semantics)
  - expert weights fetched per sample-slot with dma_gather (indices
    computed on device), weights pre-laid-out on host as one table
    row-per-(expert, contract-partition)
  - 3x3 conv as 18 accumulated fp32r matmuls (9 shifts x 2 c-blocks) per
    output half, 1x1 conv as 3x2 fp32r matmuls reading x + new features
  - experts_out written per (slot, o-block, half); out = sum of
    score-weighted expert outputs (DVE)
All five reference outputs are produced: (out, scores, idxs, query_fm,
experts_out).
"""

import sys

for _p in ('/opt/trn_rl_repo', '/root/.axon_site/_ro/trn_rl_repo'):
    if _p not in sys.path:
        try:
            import concourse  # noqa: F401
            break
        except Exception:
            sys.path.insert(0, _p)

import numpy as np

import concourse.bass as bass
import concourse.mybir as mybir
import concourse.tile as tile
from concourse import bacc
from concourse.bass_utils import run_bass_kernel_spmd

F32 = mybir.dt.float32
F32R = mybir.dt.float32r
BF16 = mybir.dt.bfloat16
FP16 = mybir.dt.float16
I32 = mybir.dt.int32
I16 = mybir.dt.int16
AX = mybir.AxisListType
OP = mybir.AluOpType
ACT = mybir.ActivationFunctionType

B, C, S, K, E = 64, 256, 28, 128, 16
NCORES = 8
BS = B // NCORES          # samples per core
HW = S * S                # 784
SP = S + 2                # 30 (padded)
HALF = HW // 2            # 392
W3_COLS = 9 * 2 * 128     # 2304
W1_COLS = 3 * 2 * 128     # 768
WROW = W3_COLS + W1_COLS  # 3072

_cache = {}


def _build_program():
    nc = bacc.Bacc()

    x_in = nc.declare_dram_parameter("x_in", [BS, C, SP * SP], FP16, isOutput=False)
    wtab = nc.declare_dram_parameter("wtab", [E * 128, WROW], BF16, isOutput=False)
    protoT = nc.declare_dram_parameter("protoT", [128, 7 * E], F32, isOutput=False)
    pn2 = nc.declare_dram_parameter("pn2", [BS, E], F32, isOutput=False)
    base16 = nc.declare_dram_parameter("base16", [128, 8], F32, isOutput=False)
    iota16 = nc.declare_dram_parameter("iota16", [BS, E], F32, isOutput=False)
    iota1e9 = nc.declare_dram_parameter("iota1e9", [BS, E], F32, isOutput=False)
    ident = nc.declare_dram_parameter("ident", [128, 128], F32, isOutput=False)
    ones1 = nc.declare_dram_parameter("ones1", [1, 128], F32, isOutput=False)
    cmean = nc.declare_dram_parameter("cmean", [128, 1], F32R, isOutput=False)

    out_d = nc.declare_dram_parameter("out_d", [BS, C, HW], F32, isOutput=True)
    scores_d = nc.declare_dram_parameter("scores_d", [BS, E], F32, isOutput=True)
    idxs_d = nc.declare_dram_parameter("idxs_d", [BS, 2], I32, isOutput=True)
    qf_d = nc.declare_dram_parameter("qf_d", [BS, HW], F32, isOutput=True)
    eo_d = nc.declare_dram_parameter("eo_d", [BS, 2, C, HW], F32, isOutput=True)

    with tile.TileContext(nc) as tc:
        with tc.tile_pool(name="const", bufs=1) as cp, \
             tc.tile_pool(name="xp", bufs=1) as xp, \
             tc.tile_pool(name="xgp", bufs=6) as xgp, \
             tc.tile_pool(name="xfp", bufs=3) as xfp, \
             tc.tile_pool(name="gate", bufs=1) as gp, \
             tc.tile_pool(name="qtp", bufs=3) as qtp:

            # ---- prefetch sample 0 gating-x before anything else ----
            xg0 = xgp.tile([128, 2, SP * SP], FP16, tag="xg16")
            for cb in range(2):
                nc.sync.dma_start(xg0[:, cb, :],
                                  x_in[0, cb * 128:(cb + 1) * 128, :])
            protoT_t = cp.tile([128, 7, E], F32)
            nc.sync.dma_start(protoT_t[:], protoT[:].rearrange("p (j e) -> p j e", j=7))
            pn2_t = cp.tile([BS, E], F32)
            nc.sync.dma_start(pn2_t[:], pn2[:])
            base16_t = cp.tile([128, 8], F32)
            nc.sync.dma_start(base16_t[:], base16[:])
            iota16_t = cp.tile([BS, E], F32)
            nc.sync.dma_start(iota16_t[:], iota16[:])
            iota1e9_t = cp.tile([BS, E], F32)
            nc.sync.dma_start(iota1e9_t[:], iota1e9[:])
            ident_t = cp.tile([128, 128], F32)
            nc.sync.dma_start(ident_t[:], ident[:])
            ones1_t = cp.tile([1, 128], F32)
            nc.sync.dma_start(ones1_t[:], ones1[:])
            cmean_t = cp.tile([128, 1], F32R)
            nc.sync.dma_start(cmean_t[:], cmean[:])

            # ---- x load into zero-padded [128, b, cb, 30*30] ----
            xbf = xp.tile([128, BS, 2, SP * SP], BF16)

            def xviewb(b, cb, row0, col0, nrows):
                """[128, nrows, 28] strided bf16 view (convs)."""
                return xbf[:, b, cb, :].rearrange("p (r c) -> p r c", r=SP)[
                    :, row0:row0 + nrows, col0:col0 + S]

            # ---- gating ----
            qf8 = gp.tile([BS, 896], F32)
            _ = None
            nc.vector.memset(qf8[:, 784:], 0.0)

            with tc.tile_pool(name="gps", bufs=2, space="PSUM") as gps, \
                 tc.tile_pool(name="gps2", bufs=1, space="PSUM") as gps2:
                for b in range(BS):
                    if b == 0:
                        xg16 = xg0
                    else:
                        xg16 = xgp.tile([128, 2, SP * SP], FP16, tag="xg16")
                        for cb in range(2):
                            nc.sync.dma_start(
                                xg16[:, cb, :],
                                x_in[b, cb * 128:(cb + 1) * 128, :])
                    xg = xfp.tile([128, 2, SP * SP], F32R, tag="xg")
                    nc.vector.tensor_copy(xg[:], xg16[:])
                    qf_ps = gps.tile([1, 1024], F32, tag="qf")
                    for half in range(2):
                        o0 = 0 if half == 0 else 512
                        out_ap = qf_ps[0:1, o0:o0 + HALF]
                        for cb in range(2):
                            rhs = xg[:, cb, :].rearrange(
                                "p (r c) -> p r c", r=SP)[
                                :, 1 + half * 14:15 + half * 14, 1:1 + S]
                            nc.tensor.matmul(out_ap, cmean_t[:, 0:1], rhs,
                                             start=(cb == 0), stop=(cb == 1))
                    nc.scalar.copy(xbf[:, b, :, :], xg16[:])
                    qftmp = qtp.tile([1, HW], F32, tag="qftmp")
                    nc.vector.tensor_copy(
                        qftmp[0:1, :].rearrange("p (c q) -> p c q", c=2),
                        qf_ps[0:1, :].rearrange("p (c q) -> p c q", c=2)[:, :, 0:HALF])
                    nc.sync.dma_start(qf8[b:b + 1, 0:HW], qftmp[0:1, :])

                nc.sync.dma_start(qf_d[:], qf8[0:BS, 0:HW])

                # qn2 = sum(qf^2); inv = 1/max(sqrt(qn2),1e-12)
                sq = gp.tile([BS, HW], F32)
                qn2 = gp.tile([BS, 1], F32)
                nc.scalar.activation(sq[:], qf8[0:BS, 0:HW], ACT.Square,
                                     accum_out=qn2[:])
                nrm = gp.tile([BS, 1], F32)
                nc.scalar.sqrt(nrm[:], qn2[:])
                nrmc = gp.tile([BS, 1], F32)
                nc.vector.tensor_scalar_max(nrmc[:], nrm[:], 1e-12)
                inv = gp.tile([BS, 1], F32)
                nc.vector.reciprocal(inv[:], nrmc[:])
                qn2i = gp.tile([BS, 1], F32)
                nc.vector.tensor_scalar(qn2i[:], qn2[:], inv[:, 0:1], inv[:, 0:1],
                                        OP.mult, OP.mult)

                # qfT via PE transpose: [8, 128] -> [128, 8] per 128-chunk
                qfT = gp.tile([128, 7, BS], F32)
                for j in range(7):
                    tp = gps.tile([128, BS], F32, tag="tp")
                    nc.tensor.transpose(tp[:], qf8[0:BS, j * 128:(j + 1) * 128],
                                        ident_t[0:BS, 0:BS])
                    nc.vector.tensor_copy(qfT[:, j, :], tp[:])

                # SS[b,e] = qf . proto_e
                ss_ps = gps2.tile([BS, E], F32, tag="ss")
                for j in range(7):
                    nc.tensor.matmul(ss_ps[:], qfT[:, j, :], protoT_t[:, j, :],
                                     start=(j == 0), stop=(j == 6))

                # d2 = qn2*inv^2 + pn2 - 2*SS*inv ; d = sqrt(max(d2,0))
                t1 = gp.tile([BS, E], F32)
                nc.vector.tensor_scalar(t1[:], ss_ps[:], inv[:, 0:1], -2.0,
                                        OP.mult, OP.mult)
                t2 = gp.tile([BS, E], F32)
                nc.vector.tensor_scalar(t2[:], t1[:], qn2i[:, 0:1], None, OP.add)
                d2 = gp.tile([BS, E], F32)
                nc.vector.tensor_tensor(d2[:], t2[:], pn2_t[:], OP.add)
                d2c = gp.tile([BS, E], F32)
                nc.vector.tensor_scalar_max(d2c[:], d2[:], 0.0)
                dd = gp.tile([BS, E], F32)
                nc.scalar.sqrt(dd[:], d2c[:])

                # softmax over 16
                mx = gp.tile([BS, 1], F32)
                nc.vector.tensor_reduce(mx[:], dd[:], axis=AX.X, op=OP.max)
                negm = gp.tile([BS, 1], F32)
                nc.vector.tensor_scalar_mul(negm[:], mx[:], -1.0)
                exps = gp.tile([BS, E], F32)
                sumexp = gp.tile([BS, 1], F32)
                nc.scalar.activation(exps[:], dd[:], ACT.Exp, bias=negm[:, 0:1],
                                     accum_out=sumexp[:])
                rsum = gp.tile([BS, 1], F32)
                nc.vector.reciprocal(rsum[:], sumexp[:])
                sc = gp.tile([BS, E], F32)
                nc.vector.tensor_scalar(sc[:], exps[:], rsum[:, 0:1], None, OP.mult)
                nc.sync.dma_start(scores_d[:], sc[:])

                # top-2 via DVE top-8 sort
                mx8 = gp.tile([BS, 8], F32)
                nc.vector.max(mx8[:], sc[:])
                ix8 = gp.tile([BS, 8], mybir.dt.uint32)
                nc.vector.max_index(ix8[:], mx8[:], sc[:])
                idxf = gp.tile([BS, 2], F32)
                nc.vector.tensor_copy(idxf[:], ix8[:, 0:2])
                idxi = gp.tile([BS, 2], I32)
                nc.vector.tensor_copy(idxi[:], ix8[:, 0:2])
                nc.sync.dma_start(idxs_d[:], idxi[:])
                msel = gp.tile([BS, 2], F32)
                nc.vector.tensor_copy(msel[:], mx8[:, 0:2])
                e128 = gp.tile([BS, 2], F32)
                nc.vector.tensor_scalar(e128[:], idxf[:], 128.0, None, OP.mult)

                # pack [8,2]+[8,2] to one partition, broadcast to 128 partitions
                bsrc = gp.tile([1, 32], F32)
                nc.sync.dma_start(
                    bsrc[0:1, 0:16].rearrange("p (b i) -> p b i", b=BS), e128[:])
                nc.sync.dma_start(
                    bsrc[0:1, 16:32].rearrange("p (b i) -> p b i", b=BS), msel[:])
                bc_ps = gps2.tile([128, 32], F32, tag="bc")
                nc.tensor.matmul(bc_ps[:], ones1_t[0:1, :], bsrc[0:1, :],
                                 start=True, stop=True)
                msb = cp.tile([128, 16], F32)
                nc.vector.tensor_copy(msb[:], bc_ps[:, 16:32])
                e128b = gp.tile([128, 16], F32)
                nc.vector.tensor_copy(e128b[:], bc_ps[:, 0:16])

                gidxf = gp.tile([128, 16, 8], F32)
                gidx = cp.tile([128, 16 * 8], I16)
                nc.vector.tensor_scalar(gidxf[:, 0, :], base16_t[:],
                                        e128b[:, 0:1], None, OP.add)
                nc.vector.tensor_copy(gidx[:, 0:8],
                                      gidxf[:, 0, :])
                for si in range(1, 16):
                    nc.vector.tensor_scalar(gidxf[:, si, :], base16_t[:],
                                            e128b[:, si:si + 1], None, OP.add)
                nc.vector.tensor_copy(gidx[:, 8:],
                                      gidxf[:, 1:, :].rearrange("p a b -> p (a b)"))

            # ---- main expert-conv loop ----
            with tc.tile_pool(name="wp", bufs=3) as wp, \
                 tc.tile_pool(name="nfp", bufs=3) as nfp, \
                 tc.tile_pool(name="eop", bufs=6) as eop, \
                 tc.tile_pool(name="accp", bufs=2) as accp, \
                 tc.tile_pool(name="cps", bufs=2, space="PSUM") as cps, \
                 tc.tile_pool(name="cps1", bufs=2, space="PSUM") as cps1:
                GROUPS = [(0, 1), (1, 1), (2, 2), (4, 4), (8, 4), (12, 4)]
                slot_map = {}
                wgroups = []
                for g, (st, sz) in enumerate(GROUPS):
                    wg = wp.tile([128, 4, WROW], BF16, tag="w")
                    nc.gpsimd.dma_gather(wg[:, 0:sz, :], wtab[:],
                                         gidx[:, st * 8:(st + sz) * 8],
                                         sz * 128, sz * 128, WROW)
                    wgroups.append(wg)
                    for k in range(sz):
                        slot_map[st + k] = (g, k)
                for b in range(BS):
                    acc = accp.tile([128, 2, 2, HALF], F32, tag="acc")
                    for i in range(2):
                        si = b * 2 + i
                        g, k = slot_map[si]
                        wsb = wgroups[g][:, k:k + 1, :]
                        nf = nfp.tile([128, 2, HALF], BF16, tag="nf")
                        ps3 = cps.tile([128, 2, 512], F32, tag="ps3")
                        j = 0
                        for dy in range(3):
                            for dx in range(3):
                                for cb in range(2):
                                    w_ap = wsb[:, 0, j * 128:(j + 1) * 128]
                                    for half in range(2):
                                        rhs = xviewb(b, cb, half * 14 + dy, dx, 14)
                                        nc.tensor.matmul(
                                            ps3[:, half, 0:HALF], w_ap,
                                            rhs, start=(j == 0), stop=(j == 17))
                                    j += 1
                        for half in range(2):
                            nc.vector.tensor_copy(nf[:, half, :],
                                                  ps3[:, half, 0:HALF])

                        for ob in range(2):
                            ps1 = cps1.tile([128, 2, 512], F32, tag="ps1")
                            for ib in range(3):
                                w0 = W3_COLS + (ib * 2 + ob) * 128
                                w_ap = wsb[:, 0, w0:w0 + 128]
                                for half in range(2):
                                    if ib < 2:
                                        rhs = xviewb(b, ib, 1 + half * 14, 1, 14)
                                    else:
                                        rhs = nf[:, half, :]
                                    nc.tensor.matmul(
                                        ps1[:, half, 0:HALF], w_ap, rhs,
                                        start=(ib == 0), stop=(ib == 2))
                            for half in range(2):
                                eo = eop.tile([128, HALF], F32, tag="eo")
                                nc.scalar.copy(eo[:], ps1[:, half, 0:HALF])
                                nc.sync.dma_start(
                                    eo_d[b, i, ob * 128:(ob + 1) * 128,
                                         half * HALF:(half + 1) * HALF], eo[:])
                                if i == 0:
                                    nc.vector.tensor_scalar(
                                        acc[:, ob, half, :], eo[:],
                                        msb[:, si:si + 1], None, OP.mult)
                                else:
                                    tmp = eop.tile([128, HALF], F32, tag="tmp")
                                    nc.vector.tensor_scalar(
                                        tmp[:], eo[:], msb[:, si:si + 1], None,
                                        OP.mult)
                                    nc.vector.tensor_tensor(
                                        acc[:, ob, half, :], acc[:, ob, half, :],
                                        tmp[:], OP.add)
                    for ob in range(2):
                        nc.sync.dma_start(
                            out_d[b, ob * 128:(ob + 1) * 128, :]
                            .rearrange("p (h q) -> p h q", h=2),
                            acc[:, ob, :, :])

    nc.finalize()
    return nc


def _host_prep(csp, ccp, proto):
    w3 = csp.transpose(0, 3, 4, 2, 1).reshape(E, 3, 3, 2, 128, K)
    w3 = w3.transpose(0, 4, 1, 2, 3, 5).reshape(E, 128, W3_COLS)
    w1 = ccp[:, :, :, 0, 0].reshape(E, 2, 128, 3, 128)
    w1 = w1.transpose(0, 4, 3, 1, 2).reshape(E, 128, W1_COLS)
    import ml_dtypes
    wtab = np.concatenate([w3, w1], axis=2).reshape(E * 128, WROW)
    wtab = np.ascontiguousarray(wtab.astype(ml_dtypes.bfloat16))

    protoT = np.zeros((128, 7, E), np.float32)
    for j in range(7):
        seg = proto[:, j * 128:min((j + 1) * 128, HW)]
        protoT[:seg.shape[1], j, :] = seg.T
    protoT = protoT.reshape(128, 7 * E)

    pn2 = np.broadcast_to((proto.astype(np.float32) ** 2).sum(axis=1)[None, :],
                          (BS, E)).copy()
    base16 = (np.arange(8)[None, :] * 16 + (np.arange(128) % 16)[:, None]).astype(
        np.float32)
    iota16 = np.broadcast_to(np.arange(E, dtype=np.float32)[None, :], (BS, E)).copy()
    iota1e9 = iota16 + 1e4
    ident = np.eye(128, dtype=np.float32)
    ones1 = np.ones((1, 128), np.float32)
    cmean = np.full((128, 1), 1.0 / C, np.float32)
    return dict(wtab=wtab, protoT=protoT, pn2=pn2, base16=base16, iota16=iota16,
                iota1e9=iota1e9, ident=ident, ones1=ones1, cmean=cmean)


def kernel(pretrained_x, x, conv_special_param, conv_channel_param, prototype,
           topk):
    assert int(topk) == 2
    x = np.asarray(x, np.float32).reshape(B, C, S, S)
    xpad = np.zeros((B, C, SP, SP), np.float32)
    xpad[:, :, 1:1 + S, 1:1 + S] = x
    xpad = xpad.reshape(B, C, SP * SP)

    consts = _host_prep(np.asarray(conv_special_param, np.float32),
                        np.asarray(conv_channel_param, np.float32),
                        np.asarray(prototype, np.float32))

    if "nc" not in _cache:
        _cache["nc"] = _build_program()
    nc = _cache["nc"]

    in_maps = []
    for c in range(NCORES):
        m = dict(consts)
        m["x_in"] = xpad.astype(np.float16)[c * BS:(c + 1) * BS]
        in_maps.append(m)

    res = run_bass_kernel_spmd(nc, in_maps, core_ids=list(range(NCORES)),
                               **_cache.get("run_kwargs", {}))
    kernel.last_results = res

    outs = res.results
    out = np.concatenate([r["out_d"] for r in outs]).reshape(B, C, S, S)
    scores = np.concatenate([r["scores_d"] for r in outs])
    idxs = np.concatenate([r["idxs_d"] for r in outs]).astype(np.int32)
    query_fm = np.concatenate([r["qf_d"] for r in outs])
    experts_out = np.concatenate([r["eo_d"] for r in outs]).reshape(B, 2, C, S, S)
    return out, scores, idxs, query_fm, experts_out


# revision 24
# speedup vs baseline: 1.1426x; 1.0075x over previous
"""Trainium2 Bass kernel for nn_CNN_MoE_v1 (moe_routing).

Strategy: data-parallel over batch across 8 NeuronCores (8 samples/core).
Per core, on device:
  - gating (fp32): channel-mean -> normalized prototype distances ->
    softmax -> top-2 (value + index, lax.top_k tie semantics)
  - expert weights fetched per sample-slot with dma_gather (indices
    computed on device), weights pre-laid-out on host as one table
    row-per-(expert, contract-partition)
  - 3x3 conv as 18 accumulated fp32r matmuls (9 shifts x 2 c-blocks) per
    output half, 1x1 conv as 3x2 fp32r matmuls reading x + new features
  - experts_out written per (slot, o-block, half); out = sum of
    score-weighted expert outputs (DVE)
All five reference outputs are produced: (out, scores, idxs, query_fm,
experts_out).
"""

import sys

for _p in ('/opt/trn_rl_repo', '/root/.axon_site/_ro/trn_rl_repo'):
    if _p not in sys.path:
        try:
            import concourse  # noqa: F401
            break
        except Exception:
            sys.path.insert(0, _p)

import numpy as np

import concourse.bass as bass
import concourse.mybir as mybir
import concourse.tile as tile
from concourse import bacc
from concourse.bass_utils import run_bass_kernel_spmd

F32 = mybir.dt.float32
F32R = mybir.dt.float32r
BF16 = mybir.dt.bfloat16
FP16 = mybir.dt.float16
I32 = mybir.dt.int32
I16 = mybir.dt.int16
AX = mybir.AxisListType
OP = mybir.AluOpType
ACT = mybir.ActivationFunctionType

B, C, S, K, E = 64, 256, 28, 128, 16
NCORES = 8
BS = B // NCORES          # samples per core
HW = S * S                # 784
SP = S + 2                # 30 (padded)
HALF = HW // 2            # 392
W3_COLS = 9 * 2 * 128     # 2304
W1_COLS = 3 * 2 * 128     # 768
WROW = W3_COLS + W1_COLS  # 3072

_cache = {}


def _build_program():
    nc = bacc.Bacc()

    x_in = nc.declare_dram_parameter("x_in", [BS, C, SP * SP], FP16, isOutput=False)
    wtab = nc.declare_dram_parameter("wtab", [E * 128, WROW], BF16, isOutput=False)
    protoT = nc.declare_dram_parameter("protoT", [128, 7 * E], F32, isOutput=False)
    pn2 = nc.declare_dram_parameter("pn2", [BS, E], F32, isOutput=False)
    base16 = nc.declare_dram_parameter("base16", [128, 8], F32, isOutput=False)
    iota16 = nc.declare_dram_parameter("iota16", [BS, E], F32, isOutput=False)
    iota1e9 = nc.declare_dram_parameter("iota1e9", [BS, E], F32, isOutput=False)
    ident = nc.declare_dram_parameter("ident", [128, 128], F32, isOutput=False)
    ones1 = nc.declare_dram_parameter("ones1", [1, 128], F32, isOutput=False)
    cmean = nc.declare_dram_parameter("cmean", [128, 1], F32R, isOutput=False)

    out_d = nc.declare_dram_parameter("out_d", [BS, C, HW], F32, isOutput=True)
    scores_d = nc.declare_dram_parameter("scores_d", [BS, E], F32, isOutput=True)
    idxs_d = nc.declare_dram_parameter("idxs_d", [BS, 2], I32, isOutput=True)
    qf_d = nc.declare_dram_parameter("qf_d", [BS, HW], F32, isOutput=True)
    eo_d = nc.declare_dram_parameter("eo_d", [BS, 2, C, HW], F32, isOutput=True)

    with tile.TileContext(nc) as tc:
        with tc.tile_pool(name="const", bufs=1) as cp, \
             tc.tile_pool(name="xp", bufs=1) as xp, \
             tc.tile_pool(name="xgp", bufs=6) as xgp, \
             tc.tile_pool(name="xfp", bufs=3) as xfp, \
             tc.tile_pool(name="gate", bufs=1) as gp, \
             tc.tile_pool(name="qtp", bufs=3) as qtp:

            # ---- prefetch sample 0 gating-x before anything else ----
            xg0 = xgp.tile([128, 2, SP * SP], FP16, tag="xg16")
            for cb in range(2):
                nc.sync.dma_start(xg0[:, cb, :],
                                  x_in[0, cb * 128:(cb + 1) * 128, :])
            protoT_t = cp.tile([128, 7, E], F32)
            nc.sync.dma_start(protoT_t[:], protoT[:].rearrange("p (j e) -> p j e", j=7))
            pn2_t = cp.tile([BS, E], F32)
            nc.sync.dma_start(pn2_t[:], pn2[:])
            base16_t = cp.tile([128, 8], F32)
            nc.sync.dma_start(base16_t[:], base16[:])
            iota16_t = cp.tile([BS, E], F32)
            nc.sync.dma_start(iota16_t[:], iota16[:])
            iota1e9_t = cp.tile([BS, E], F32)
            nc.sync.dma_start(iota1e9_t[:], iota1e9[:])
            ident_t = cp.tile([128, 128], F32)
            nc.sync.dma_start(ident_t[:], ident[:])
            ones1_t = cp.tile([1, 128], F32)
            nc.sync.dma_start(ones1_t[:], ones1[:])
            cmean_t = cp.tile([128, 1], F32R)
            nc.sync.dma_start(cmean_t[:], cmean[:])

            # ---- x load into zero-padded [128, b, cb, 30*30] ----
            xbf = xp.tile([128, BS, 2, SP * SP], BF16)

            def xviewb(b, cb, row0, col0, nrows):
                """[128, nrows, 28] strided bf16 view (convs)."""
                return xbf[:, b, cb, :].rearrange("p (r c) -> p r c", r=SP)[
                    :, row0:row0 + nrows, col0:col0 + S]

            # ---- gating ----
            qf8 = gp.tile([BS, 896], F32)
            _ = None
            nc.vector.memset(qf8[:, 784:], 0.0)

            with tc.tile_pool(name="gps", bufs=2, space="PSUM") as gps, \
                 tc.tile_pool(name="gps2", bufs=1, space="PSUM") as gps2:
                for b in range(BS):
                    if b == 0:
                        xg16 = xg0
                    else:
                        xg16 = xgp.tile([128, 2, SP * SP], FP16, tag="xg16")
                        for cb in range(2):
                            nc.sync.dma_start(
                                xg16[:, cb, :],
                                x_in[b, cb * 128:(cb + 1) * 128, :])
                    xg = xfp.tile([128, 2, SP * SP], F32R, tag="xg")
                    for cb in range(2):
                        nc.vector.tensor_copy(xg[:, cb, :], xg16[:, cb, :])
                    qf_ps = gps.tile([1, 1024], F32, tag="qf")
                    for half in range(2):
                        o0 = 0 if half == 0 else 512
                        out_ap = qf_ps[0:1, o0:o0 + HALF]
                        for cb in range(2):
                            rhs = xg[:, cb, :].rearrange(
                                "p (r c) -> p r c", r=SP)[
                                :, 1 + half * 14:15 + half * 14, 1:1 + S]
                            nc.tensor.matmul(out_ap, cmean_t[:, 0:1], rhs,
                                             start=(cb == 0), stop=(cb == 1))
                    nc.scalar.copy(xbf[:, b, :, :], xg16[:])
                    qftmp = qtp.tile([1, HW], F32, tag="qftmp")
                    nc.vector.tensor_copy(
                        qftmp[0:1, :].rearrange("p (c q) -> p c q", c=2),
                        qf_ps[0:1, :].rearrange("p (c q) -> p c q", c=2)[:, :, 0:HALF])
                    nc.sync.dma_start(qf8[b:b + 1, 0:HW], qftmp[0:1, :])

                nc.sync.dma_start(qf_d[:], qf8[0:BS, 0:HW])

                # qn2 = sum(qf^2); inv = 1/max(sqrt(qn2),1e-12)
                sq = gp.tile([BS, HW], F32)
                qn2 = gp.tile([BS, 1], F32)
                nc.scalar.activation(sq[:], qf8[0:BS, 0:HW], ACT.Square,
                                     accum_out=qn2[:])
                nrm = gp.tile([BS, 1], F32)
                nc.scalar.sqrt(nrm[:], qn2[:])
                nrmc = gp.tile([BS, 1], F32)
                nc.vector.tensor_scalar_max(nrmc[:], nrm[:], 1e-12)
                inv = gp.tile([BS, 1], F32)
                nc.vector.reciprocal(inv[:], nrmc[:])
                qn2i = gp.tile([BS, 1], F32)
                nc.vector.tensor_scalar(qn2i[:], qn2[:], inv[:, 0:1], inv[:, 0:1],
                                        OP.mult, OP.mult)

                # qfT via PE transpose: [8, 128] -> [128, 8] per 128-chunk
                qfT = gp.tile([128, 7, BS], F32)
                for j in range(7):
                    tp = gps.tile([128, BS], F32, tag="tp")
                    nc.tensor.transpose(tp[:], qf8[0:BS, j * 128:(j + 1) * 128],
                                        ident_t[0:BS, 0:BS])
                    nc.vector.tensor_copy(qfT[:, j, :], tp[:])

                # SS[b,e] = qf . proto_e
                ss_ps = gps2.tile([BS, E], F32, tag="ss")
                for j in range(7):
                    nc.tensor.matmul(ss_ps[:], qfT[:, j, :], protoT_t[:, j, :],
                                     start=(j == 0), stop=(j == 6))

                # d2 = qn2*inv^2 + pn2 - 2*SS*inv ; d = sqrt(max(d2,0))
                t1 = gp.tile([BS, E], F32)
                nc.vector.tensor_scalar(t1[:], ss_ps[:], inv[:, 0:1], -2.0,
                                        OP.mult, OP.mult)
                t2 = gp.tile([BS, E], F32)
                nc.vector.tensor_scalar(t2[:], t1[:], qn2i[:, 0:1], None, OP.add)
                d2 = gp.tile([BS, E], F32)
                nc.vector.tensor_tensor(d2[:], t2[:], pn2_t[:], OP.add)
                d2c = gp.tile([BS, E], F32)
                nc.vector.tensor_scalar_max(d2c[:], d2[:], 0.0)
                dd = gp.tile([BS, E], F32)
                nc.scalar.sqrt(dd[:], d2c[:])

                # softmax over 16
                mx = gp.tile([BS, 1], F32)
                nc.vector.tensor_reduce(mx[:], dd[:], axis=AX.X, op=OP.max)
                negm = gp.tile([BS, 1], F32)
                nc.vector.tensor_scalar_mul(negm[:], mx[:], -1.0)
                exps = gp.tile([BS, E], F32)
                sumexp = gp.tile([BS, 1], F32)
                nc.scalar.activation(exps[:], dd[:], ACT.Exp, bias=negm[:, 0:1],
                                     accum_out=sumexp[:])
                rsum = gp.tile([BS, 1], F32)
                nc.vector.reciprocal(rsum[:], sumexp[:])
                sc = gp.tile([BS, E], F32)
                nc.vector.tensor_scalar(sc[:], exps[:], rsum[:, 0:1], None, OP.mult)
                nc.sync.dma_start(scores_d[:], sc[:])

                # top-2 via DVE top-8 sort
                mx8 = gp.tile([BS, 8], F32)
                nc.vector.max(mx8[:], sc[:])
                ix8 = gp.tile([BS, 8], mybir.dt.uint32)
                nc.vector.max_index(ix8[:], mx8[:], sc[:])
                idxf = gp.tile([BS, 2], F32)
                nc.vector.tensor_copy(idxf[:], ix8[:, 0:2])
                idxi = gp.tile([BS, 2], I32)
                nc.vector.tensor_copy(idxi[:], ix8[:, 0:2])
                nc.sync.dma_start(idxs_d[:], idxi[:])
                msel = gp.tile([BS, 2], F32)
                nc.vector.tensor_copy(msel[:], mx8[:, 0:2])
                e128 = gp.tile([BS, 2], F32)
                nc.vector.tensor_scalar(e128[:], idxf[:], 128.0, None, OP.mult)

                # pack [8,2]+[8,2] to one partition, broadcast to 128 partitions
                bsrc = gp.tile([1, 32], F32)
                nc.sync.dma_start(
                    bsrc[0:1, 0:16].rearrange("p (b i) -> p b i", b=BS), e128[:])
                nc.sync.dma_start(
                    bsrc[0:1, 16:32].rearrange("p (b i) -> p b i", b=BS), msel[:])
                bc_ps = gps2.tile([128, 32], F32, tag="bc")
                nc.tensor.matmul(bc_ps[:], ones1_t[0:1, :], bsrc[0:1, :],
                                 start=True, stop=True)
                msb = cp.tile([128, 16], F32)
                nc.vector.tensor_copy(msb[:], bc_ps[:, 16:32])
                e128b = gp.tile([128, 16], F32)
                nc.vector.tensor_copy(e128b[:], bc_ps[:, 0:16])

                gidxf = gp.tile([128, 16, 8], F32)
                gidx = cp.tile([128, 16 * 8], I16)
                nc.vector.tensor_scalar(gidxf[:, 0, :], base16_t[:],
                                        e128b[:, 0:1], None, OP.add)
                nc.vector.tensor_copy(gidx[:, 0:8],
                                      gidxf[:, 0, :])
                for si in range(1, 16):
                    nc.vector.tensor_scalar(gidxf[:, si, :], base16_t[:],
                                            e128b[:, si:si + 1], None, OP.add)
                nc.vector.tensor_copy(gidx[:, 8:],
                                      gidxf[:, 1:, :].rearrange("p a b -> p (a b)"))

            # ---- main expert-conv loop ----
            with tc.tile_pool(name="wp", bufs=3) as wp, \
                 tc.tile_pool(name="nfp", bufs=3) as nfp, \
                 tc.tile_pool(name="eop", bufs=6) as eop, \
                 tc.tile_pool(name="accp", bufs=2) as accp, \
                 tc.tile_pool(name="cps", bufs=2, space="PSUM") as cps, \
                 tc.tile_pool(name="cps1", bufs=2, space="PSUM") as cps1:
                GROUPS = [(0, 1), (1, 1), (2, 2), (4, 4), (8, 4), (12, 4)]
                slot_map = {}
                wgroups = []
                for g, (st, sz) in enumerate(GROUPS):
                    wg = wp.tile([128, 4, WROW], BF16, tag="w")
                    nc.gpsimd.dma_gather(wg[:, 0:sz, :], wtab[:],
                                         gidx[:, st * 8:(st + sz) * 8],
                                         sz * 128, sz * 128, WROW)
                    wgroups.append(wg)
                    for k in range(sz):
                        slot_map[st + k] = (g, k)
                for b in range(BS):
                    acc = accp.tile([128, 2, 2, HALF], F32, tag="acc")
                    for i in range(2):
                        si = b * 2 + i
                        g, k = slot_map[si]
                        wsb = wgroups[g][:, k:k + 1, :]
                        nf = nfp.tile([128, 2, HALF], BF16, tag="nf")
                        ps3 = cps.tile([128, 2, 512], F32, tag="ps3")
                        j = 0
                        for dy in range(3):
                            for dx in range(3):
                                for cb in range(2):
                                    w_ap = wsb[:, 0, j * 128:(j + 1) * 128]
                                    for half in range(2):
                                        rhs = xviewb(b, cb, half * 14 + dy, dx, 14)
                                        nc.tensor.matmul(
                                            ps3[:, half, 0:HALF], w_ap,
                                            rhs, start=(j == 0), stop=(j == 17))
                                    j += 1
                        for half in range(2):
                            nc.vector.tensor_copy(nf[:, half, :],
                                                  ps3[:, half, 0:HALF])

                        for ob in range(2):
                            ps1 = cps1.tile([128, 2, 512], F32, tag="ps1")
                            for ib in range(3):
                                w0 = W3_COLS + (ib * 2 + ob) * 128
                                w_ap = wsb[:, 0, w0:w0 + 128]
                                for half in range(2):
                                    if ib < 2:
                                        rhs = xviewb(b, ib, 1 + half * 14, 1, 14)
                                    else:
                                        rhs = nf[:, half, :]
                                    nc.tensor.matmul(
                                        ps1[:, half, 0:HALF], w_ap, rhs,
                                        start=(ib == 0), stop=(ib == 2))
                            for half in range(2):
                                eo = eop.tile([128, HALF], F32, tag="eo")
                                if si >= 14:
                                    nc.vector.tensor_copy(eo[:],
                                                          ps1[:, half, 0:HALF])
                                else:
                                    nc.scalar.copy(eo[:], ps1[:, half, 0:HALF])
                                nc.sync.dma_start(
                                    eo_d[b, i, ob * 128:(ob + 1) * 128,
                                         half * HALF:(half + 1) * HALF], eo[:])
                                if i == 0:
                                    nc.vector.tensor_scalar(
                                        acc[:, ob, half, :], eo[:],
                                        msb[:, si:si + 1], None, OP.mult)
                                else:
                                    tmp = eop.tile([128, HALF], F32, tag="tmp")
                                    nc.vector.tensor_scalar(
                                        tmp[:], eo[:], msb[:, si:si + 1], None,
                                        OP.mult)
                                    nc.vector.tensor_tensor(
                                        acc[:, ob, half, :], acc[:, ob, half, :],
                                        tmp[:], OP.add)
                    for ob in range(2):
                        nc.sync.dma_start(
                            out_d[b, ob * 128:(ob + 1) * 128, :]
                            .rearrange("p (h q) -> p h q", h=2),
                            acc[:, ob, :, :])

    nc.finalize()
    return nc


def _host_prep(csp, ccp, proto):
    w3 = csp.transpose(0, 3, 4, 2, 1).reshape(E, 3, 3, 2, 128, K)
    w3 = w3.transpose(0, 4, 1, 2, 3, 5).reshape(E, 128, W3_COLS)
    w1 = ccp[:, :, :, 0, 0].reshape(E, 2, 128, 3, 128)
    w1 = w1.transpose(0, 4, 3, 1, 2).reshape(E, 128, W1_COLS)
    import ml_dtypes
    wtab = np.concatenate([w3, w1], axis=2).reshape(E * 128, WROW)
    wtab = np.ascontiguousarray(wtab.astype(ml_dtypes.bfloat16))

    protoT = np.zeros((128, 7, E), np.float32)
    for j in range(7):
        seg = proto[:, j * 128:min((j + 1) * 128, HW)]
        protoT[:seg.shape[1], j, :] = seg.T
    protoT = protoT.reshape(128, 7 * E)

    pn2 = np.broadcast_to((proto.astype(np.float32) ** 2).sum(axis=1)[None, :],
                          (BS, E)).copy()
    base16 = (np.arange(8)[None, :] * 16 + (np.arange(128) % 16)[:, None]).astype(
        np.float32)
    iota16 = np.broadcast_to(np.arange(E, dtype=np.float32)[None, :], (BS, E)).copy()
    iota1e9 = iota16 + 1e4
    ident = np.eye(128, dtype=np.float32)
    ones1 = np.ones((1, 128), np.float32)
    cmean = np.full((128, 1), 1.0 / C, np.float32)
    return dict(wtab=wtab, protoT=protoT, pn2=pn2, base16=base16, iota16=iota16,
                iota1e9=iota1e9, ident=ident, ones1=ones1, cmean=cmean)


def kernel(pretrained_x, x, conv_special_param, conv_channel_param, prototype,
           topk):
    assert int(topk) == 2
    x = np.asarray(x, np.float32).reshape(B, C, S, S)
    xpad = np.zeros((B, C, SP, SP), np.float32)
    xpad[:, :, 1:1 + S, 1:1 + S] = x
    xpad = xpad.reshape(B, C, SP * SP)

    consts = _host_prep(np.asarray(conv_special_param, np.float32),
                        np.asarray(conv_channel_param, np.float32),
                        np.asarray(prototype, np.float32))

    if "nc" not in _cache:
        _cache["nc"] = _build_program()
    nc = _cache["nc"]

    in_maps = []
    for c in range(NCORES):
        m = dict(consts)
        m["x_in"] = xpad.astype(np.float16)[c * BS:(c + 1) * BS]
        in_maps.append(m)

    res = run_bass_kernel_spmd(nc, in_maps, core_ids=list(range(NCORES)),
                               **_cache.get("run_kwargs", {}))
    kernel.last_results = res

    outs = res.results
    out = np.concatenate([r["out_d"] for r in outs]).reshape(B, C, S, S)
    scores = np.concatenate([r["scores_d"] for r in outs])
    idxs = np.concatenate([r["idxs_d"] for r in outs]).astype(np.int32)
    query_fm = np.concatenate([r["qf_d"] for r in outs])
    experts_out = np.concatenate([r["eo_d"] for r in outs]).reshape(B, 2, C, S, S)
    return out, scores, idxs, query_fm, experts_out
